# revision 42
# baseline (speedup 1.0000x reference)
"""Sliding-window GQA causal self-attention for Trainium2, 8 NeuronCores.

Sharding: 8 cores = 4 batches x 2 head-shards. Each core handles one batch
and 2 of the 4 KV groups (8 of 16 Q heads). Core computes a full [C, T]
partial of the output projection; host sums the two shards per batch.

Design (v2, natural-layout stage A, bf16 operands):
  Projections run "natural" (tokens on partitions): per 128-token block j,
  q_ps[t, 512], k/v/gate packed in one PSUM tile. RoPE becomes free-dim
  32-shifts (no partition swaps); rms uses rotation invariance (computed
  from pre-rope PSUM via square+reduce); rstd_k folds into the PSUM->SBUF
  copy (Act copy with per-partition scale), rstd_q (with the 1.2*1.2/8
  score scale) folds into the per-head Act copies. q/k transpose via PE
  into bf16 PSUM, Pool copies them out.

  Attention: per (h-half, r): scores^T via bf16 QK matmuls, Exp on Act,
  band-edge masks via DVE multiplies with precomputed triangle masks.
  V carries a ones-column so softmax denominators fall out of the PV
  matmul (row 64 of y_ps). Normalize: DVE reciprocal -> rank-1 broadcast
  matmul -> DVE/Pool multiplies into yTf (bf16). Output projection per
  h-half interleaves with the other half's attention on the PE.
"""
import numpy as np

B, T, C = 4, 1024, 1024
H, HKV, D = 16, 4, 64
REP = H // HKV
WINDOW = 256
GATE_CH = 12
NCORES = 8
EPS = float(np.finfo(np.float32).eps)
QK_SCALE = 1.2 * 1.2 / 8.0  # two rms gains (1.2 each) * 1/sqrt(D)

_CACHE = {}


def _build_program(debug=False, reps=1):
    from contextlib import ExitStack
    import concourse.bass as bass
    import concourse.tile as tile
    from concourse import bacc, mybir
    from concourse.masks import make_identity

    f32 = mybir.dt.float32
    f32r = mybir.dt.float32r
    bf16 = mybir.dt.bfloat16
    ts = bass.ts

    nc = bacc.Bacc("TRN2", target_bir_lowering=False, debug=False,
                   enable_asserts=True, num_devices=NCORES)

    def din(name, shape, dt=bf16):
        return nc.dram_tensor(name, shape, dt, kind="ExternalInput").ap()

    # host-prearranged layouts (see _prep_core_inputs)
    xtn = din("xtn", [128, 8, 8, 128])    # [c%128, c//128, j, t%128] = x^T
    wqn = din("wqn", [128, 8, 512])       # [c%128, c//128, r*128+gg*64+d]
    wkn = din("wkn", [128, 8, 128])       # [c%128, c//128, gg*64+d]
    wvn = din("wvn", [128, 8, 128])
    won = din("won", [128, 4, 1024])      # [gg*64+d, r, c]
    wgn = din("wgn", [16, 2])             # zero-padded 12->16 gate rows
    ve3n = din("ve3n", [128, 8, 2, 64])   # [t%128, j, gg, d] = 3*ve
    cpat = din("cpat", [128, 8, 64])      # cos[t, d%32]
    spat = din("spat", [128, 8, 64])      # -sin[t,d] / +sin[t,d-32] halves
    mlo = din("mlo", [128, 128])          # causal edge: 1 if col >= row
    mhi = din("mhi", [128, 128])          # window edge: 1 if col <= row
    outT = nc.dram_tensor("out_t", [C, T], bf16, kind="ExternalOutput").ap()
    dbg = {}
    if debug:
        for nm, shp in [("d_qTf", [128, 4, T]), ("d_kTf", [128, T]),
                        ("d_v", [128, 8, 2, 65]), ("d_yTf", [128, 4, T]),
                        ("d_ks", [128, 8, 2, 64]), ("d_kr", [128, 8, 2, 64]),
                        ("d_rsk", [128, 8, 2]), ("d_msqk", [128, 8, 2]),
                        ("d_kraw", [128, 8, 2, 64]), ("d_k2", [128, 8, 2, 64]),
                        ("d_rs", [1, 2, 512]), ("d_rbs", [64, 2, 512]),
                        ("d_sums", [1, 2, 512])]:
            dt = f32 if nm in ("d_rsk", "d_msqk", "d_rs", "d_rbs",
                               "d_sums") else bf16
            dbg[nm] = nc.dram_tensor(nm, shp, dt, kind="ExternalOutput").ap()

    Exp = mybir.ActivationFunctionType.Exp
    Square = mybir.ActivationFunctionType.Square
    Sqrt = mybir.ActivationFunctionType.Sqrt
    Copy = mybir.ActivationFunctionType.Copy
    add_op = mybir.AluOpType.add
    mult_op = mybir.AluOpType.mult

    with tile.TileContext(nc) as tc:
     for _rep in range(reps):
      with ExitStack() as ctx:
        sing = ctx.enter_context(tc.tile_pool(name="sing", bufs=1))

        # ---------- persistent tiles + loads ----------
        # SP queue: xt0/wk/wv first so tblock-0 k/v matmuls start ASAP;
        # Act queue takes the rope/gate tables + wo (needed later).
        xt_sb = [sing.tile([128, 8, 128], bf16, name=f"xt{j}") for j in range(8)]
        nc.sync.dma_start(xt_sb[0][:], xtn[:, :, 0, :])
        wk_sb = sing.tile([128, 8, 128], bf16, name="wk_sb")
        nc.sync.dma_start(wk_sb[:], wkn[:])
        wv_sb = sing.tile([128, 8, 128], bf16, name="wv_sb")
        nc.sync.dma_start(wv_sb[:], wvn[:])
        wg_sb = sing.tile([16, 2], bf16, name="wg_sb")
        nc.sync.dma_start(wg_sb[:], wgn[:])
        nc.sync.dma_start(xt_sb[1][:], xtn[:, :, 1, :])
        wq_sb = sing.tile([128, 8, 512], bf16, name="wq_sb")
        nc.sync.dma_start(wq_sb[:, 0:4, :], wqn[:, 0:4, :])
        nc.sync.dma_start(xt_sb[2][:], xtn[:, :, 2, :])
        nc.sync.dma_start(wq_sb[:, 4:8, :], wqn[:, 4:8, :])
        for j in range(3, 8):
            nc.sync.dma_start(xt_sb[j][:], xtn[:, :, j, :])
        cpat_sb = sing.tile([128, 8, 64], bf16, name="cpat_sb")
        nc.scalar.dma_start(cpat_sb[:], cpat[:])
        spat_sb = sing.tile([128, 8, 64], bf16, name="spat_sb")
        nc.scalar.dma_start(spat_sb[:], spat[:])
        ve_sb = sing.tile([128, 8, 2, 64], bf16, name="ve_sb")
        nc.scalar.dma_start(ve_sb[:], ve3n[:])
        mlo_sb = sing.tile([128, 128], bf16, name="mlo_sb")
        nc.scalar.dma_start(mlo_sb[:], mlo[:])
        mhi_sb = sing.tile([128, 128], bf16, name="mhi_sb")
        nc.scalar.dma_start(mhi_sb[:], mhi[:])
        # wo only needed by stage C
        wo_sb = sing.tile([128, 4, 1024], bf16, name="wo_sb")
        nc.scalar.dma_start(wo_sb[:], won[:])

        ident = sing.tile([128, 128], bf16, name="ident")
        make_identity(nc, ident[:])

        epsk_sb = sing.tile([128, 1], f32, name="epsk_sb")
        nc.vector.memset(epsk_sb[:], EPS)
        epsq_sb = sing.tile([128, 1], f32, name="epsq_sb")
        nc.vector.memset(epsq_sb[:], EPS / (QK_SCALE * QK_SCALE))
        qTf = sing.tile([128, 4, T], bf16, name="qTf")   # [(gg,d), r, t]
        kTf = sing.tile([128, T], bf16, name="kTf")      # [(gg,d), t]
        v_sb = sing.tile([128, 8, 2, 65], bf16, name="v_sb")  # [t%128, j, gg, d|1]
        yTf = sing.tile([128, 4, T], bf16, name="yTf")
        nc.vector.memset(v_sb[:, :, :, 64:65], 1.0)

        # ================= Stage A: projections / rope / rms / gate =========
        with tc.tile_pool(name="stA", bufs=2) as stA, \
             tc.tile_pool(name="pA_", bufs=2, space="PSUM") as pA_:
            for j in range(8):
                xt = xt_sb[j]
                # ---- k / v / gate packed in one PSUM bank
                # k, v, gate groups must be sequential: interleaving two
                # accumulation groups in one PSUM bank loses contributions.
                kvg = pA_.tile([128, 258], f32, name="kvg", tag="kvg")
                for kc in range(8):
                    nc.tensor.matmul(kvg[:, 0:128], xt[:, kc, :],
                                     wk_sb[:, kc, :], start=(kc == 0),
                                     stop=(kc == 7), skip_group_check=True)
                for kc in range(8):
                    nc.tensor.matmul(kvg[:, 128:256], xt[:, kc, :],
                                     wv_sb[:, kc, :], start=(kc == 0),
                                     stop=(kc == 7), skip_group_check=True)
                nc.tensor.matmul(kvg[:, 256:258], xt[0:16, 0, :], wg_sb[:],
                                 start=True, stop=True, skip_group_check=True)
                # ---- q projection
                q_ps = pA_.tile([128, 8, 64], f32, name="q_ps", tag="qps")
                for kc in range(8):
                    nc.tensor.matmul(q_ps[:].rearrange("p h d -> p (h d)"),
                                     xt[:, kc, :], wq_sb[:, kc, :],
                                     start=(kc == 0), stop=(kc == 7))

                kk = kvg[:, 0:128].rearrange("p (g d) -> p g d", g=2)
                # ---- k rms (pre-rope; rotation preserves norms).
                # Squares on Act (PSUM ops may read only one PSUM input).
                k2 = stA.tile([128, 2, 64], bf16, name="k2", tag="k2")
                nc.scalar.activation(k2[:], kk, Square)
                msqk = stA.tile([128, 2], f32, name="msqk", tag="msqk")
                nc.vector.tensor_reduce(msqk[:], k2[:], mybir.AxisListType.X,
                                        add_op)
                sqk = stA.tile([128, 2], f32, name="sqk", tag="sqk")
                nc.scalar.activation(sqk[:], msqk[:], Sqrt, scale=1.0 / D,
                                     bias=epsk_sb[:])
                rsk = stA.tile([128, 2], f32, name="rsk", tag="rsk")
                nc.vector.reciprocal(rsk[:], sqk[:])
                # ---- k: PSUM->SBUF with rstd_k folded into the copy
                ks = stA.tile([128, 2, 64], bf16, name="ks", tag="ks")
                for gg in range(2):
                    nc.scalar.activation(ks[:, gg, :], kk[:, gg, :], Copy,
                                         scale=rsk[:, gg:gg + 1])
                # ---- k rope (Pool): kr = ks*cpat + shift32(ks*spat)
                spat_j2 = spat_sb[:, j:j + 1, :].broadcast_to([128, 2, 64])
                cpat_j2 = cpat_sb[:, j:j + 1, :].broadcast_to([128, 2, 64])
                ku = stA.tile([128, 2, 64], bf16, name="ku", tag="ku")
                nc.vector.tensor_mul(ku[:], ks[:], spat_j2)
                kr = stA.tile([128, 2, 64], bf16, name="kr", tag="kr")
                nc.vector.tensor_mul(kr[:], ks[:], cpat_j2)
                nc.vector.tensor_add(kr[:, :, 0:32], kr[:, :, 0:32],
                                     ku[:, :, 32:64])
                nc.vector.tensor_add(kr[:, :, 32:64], kr[:, :, 32:64],
                                     ku[:, :, 0:32])
                if debug:
                    nc.sync.dma_start(dbg["d_ks"][:, j, :, :], ks[:])
                    nc.sync.dma_start(dbg["d_kr"][:, j, :, :], kr[:])
                    nc.sync.dma_start(dbg["d_rsk"][:, j, :], rsk[:])
                    nc.sync.dma_start(dbg["d_msqk"][:, j, :], msqk[:])
                    kraw = stA.tile([128, 2, 64], bf16, name="kraw", tag="kraw")
                    nc.vector.tensor_copy(kraw[:], kk)
                    nc.sync.dma_start(dbg["d_kraw"][:, j, :, :], kraw[:])
                    nc.sync.dma_start(dbg["d_k2"][:, j, :, :], k2[:])

                # ---- v + gate*ve (gate = 3*sigmoid, the 3 is folded in ve3).
                # g = x[:, :12] @ Wg^T is tiny (|g| < ~0.5 since Wg ~ 0.02),
                # so sigmoid(g) ~= 0.5 + g*(0.25 - g^2/48), err < 2e-4.
                gg2 = stA.tile([128, 2], f32, name="gg2", tag="gg2")
                nc.scalar.activation(gg2[:], kvg[:, 256:258], Square)
                nc.vector.tensor_scalar(gg2[:], gg2[:], -1.0 / 48.0, 0.25,
                                        mult_op, add_op)
                sg = stA.tile([128, 2], f32, name="sg", tag="sg")
                nc.vector.tensor_mul(sg[:], gg2[:], kvg[:, 256:258])
                nc.vector.tensor_scalar_add(sg[:], sg[:], 0.5)
                gve = stA.tile([128, 2, 64], bf16, name="gve", tag="gve")
                for gg in range(2):
                    nc.gpsimd.tensor_scalar_mul(gve[:, gg, :],
                                                ve_sb[:, j, gg, :],
                                                sg[:, gg:gg + 1])
                nc.vector.tensor_add(
                    v_sb[:, j, :, 0:64],
                    kvg[:, 128:256].rearrange("p (g d) -> p g d", g=2), gve[:])

                # ---- q rms (pre-rope), rstd_q folded with QK_SCALE
                q2 = stA.tile([128, 8, 64], bf16, name="q2", tag="q2")
                nc.scalar.activation(q2[:], q_ps[:], Square)
                msqq = stA.tile([128, 8], f32, name="msqq", tag="msqq")
                nc.vector.tensor_reduce(msqq[:], q2[:], mybir.AxisListType.X,
                                        add_op)
                sqq = stA.tile([128, 8], f32, name="sqq", tag="sqq")
                nc.scalar.activation(sqq[:], msqq[:], Sqrt,
                                     scale=1.0 / (D * QK_SCALE * QK_SCALE),
                                     bias=epsq_sb[:])
                rsq = stA.tile([128, 8], f32, name="rsq", tag="rsq")
                nc.vector.reciprocal(rsq[:], sqq[:])
                # ---- q: PSUM->SBUF bf16 with per-head rstd_q scale
                qs = stA.tile([128, 8, 64], bf16, name="qs", tag="qs")
                for h8 in range(8):
                    nc.scalar.activation(qs[:, h8, :], q_ps[:, h8, :], Copy,
                                         scale=rsq[:, h8:h8 + 1])
                # ---- q rope (DVE, all-SBUF bf16)
                spat_j8 = spat_sb[:, j:j + 1, :].broadcast_to([128, 8, 64])
                cpat_j8 = cpat_sb[:, j:j + 1, :].broadcast_to([128, 8, 64])
                qu = stA.tile([128, 8, 64], bf16, name="qu", tag="qu")
                nc.vector.tensor_mul(qu[:], qs[:], spat_j8)
                qr = stA.tile([128, 8, 64], bf16, name="qr", tag="qr")
                nc.vector.tensor_mul(qr[:], qs[:], cpat_j8)
                nc.vector.tensor_add(qr[:, :, 0:32], qr[:, :, 0:32],
                                     qu[:, :, 32:64])
                nc.vector.tensor_add(qr[:, :, 32:64], qr[:, :, 32:64],
                                     qu[:, :, 0:32])

                # ---- transposes into bf16 PSUM, Pool copies out
                tp = pA_.tile([128, 640], bf16, name="tp", tag="tp")
                for r in range(4):
                    nc.tensor.transpose(
                        tp[:, ts(r, 128)],
                        qr[:, 2 * r:2 * r + 2, :].rearrange("p g d -> p (g d)"),
                        ident[:])
                nc.tensor.transpose(
                    tp[:, 512:640], kr[:].rearrange("p g d -> p (g d)"),
                    ident[:])
                nc.scalar.activation(
                    qTf[:, :, ts(j, 128)],
                    tp[:, 0:512].rearrange("p (r t) -> p r t", r=4), Copy)
                nc.vector.tensor_copy(kTf[:, ts(j, 128)], tp[:, 512:640])

        # ================= Stage B + C: attention + out projection ==========
        with tc.tile_pool(name="stB", bufs=2) as stB, \
             tc.tile_pool(name="pB_", bufs=1, space="PSUM") as pB_:
            pkeep = {(r, j): stB.tile([128, 2, 384], bf16, name=f"pk{r}{j}",
                                      tag=f"pk{r}{j}", bufs=1)
                     for r in range(4) for j in (2, 3)}
            ccount = 0
            for h in range(2):
                tsl = slice(512 * h, 512 * h + 512)
                jlist = list(range(0, 4)) if h == 0 else list(range(2, 8))
                for r in range(4):
                    y_ps = pB_.tile([65, 2, 512], f32, name="y_ps", tag="yps")
                    first = True
                    for j in jlist:
                        w = min(384, T - 128 * j)
                        if h == 1 and j in (2, 3):
                            p2 = pkeep[(r, j)]  # cached from h == 0
                        else:
                            p2 = (pkeep[(r, j)] if j in (2, 3) else
                                  stB.tile([128, 2, 384], bf16, name="p2",
                                           tag="p2", bufs=3))
                            sc2 = pB_.tile([128, 2, 512], f32, name="sc2",
                                           tag="sc", bufs=2)
                            for gg in range(2):
                                nc.tensor.matmul(
                                    sc2[:, gg, 0:w],
                                    kTf[ts(gg, 64), ts(j, 128)],
                                    qTf[ts(gg, 64), r, 128 * j:128 * j + w],
                                    start=True, stop=True)
                            nc.scalar.activation(p2[:, :, 0:w], sc2[:, :, 0:w],
                                                 Exp)
                            # band edges: causal lower, window upper triangle
                            nc.vector.tensor_mul(
                                p2[:, :, 0:128], p2[:, :, 0:128],
                                mlo_sb[:].unsqueeze(1).broadcast_to([128, 2, 128]))
                            if w > 256:
                                nc.vector.tensor_mul(
                                    p2[:, :, 256:384], p2[:, :, 256:384],
                                    mhi_sb[:].unsqueeze(1).broadcast_to([128, 2, 128]))
                        a = max(128 * j, 512 * h)
                        b = min(128 * j + w, 512 * h + 512)
                        n0, nn = a - 128 * j, b - a
                        for gg in range(2):
                            nc.tensor.matmul(
                                y_ps[:, gg, a - 512 * h:b - 512 * h],
                                v_sb[:, j, gg, :], p2[:, gg, n0:n0 + nn],
                                start=first, stop=(j == jlist[-1]),
                                skip_group_check=True)
                        first = False
                    # normalize: 1/sums (row 64) -> rank-1 broadcast -> apply
                    su = stB.tile([1, 2, 512], f32, name="su", tag="su")
                    nc.scalar.activation(su[:], y_ps[64:65, :, :], Copy)
                    rs = stB.tile([1, 2, 512], f32, name="rs", tag="rs")
                    nc.vector.reciprocal_approx_fast(rs[:], su[:])
                    rbs = stB.tile([64, 2, 512], f32, name="rbs", tag="rbs")
                    nc.gpsimd.partition_broadcast(rbs[:], rs[:], channels=64)
                    if debug and h == 0 and r == 0:
                        sums_sb = stB.tile([1, 2, 512], f32, name="sums_sb",
                                           tag="sums_sb")
                        nc.vector.tensor_copy(sums_sb[:], y_ps[64:65, :, :])
                        nc.sync.dma_start(dbg["d_sums"][:], sums_sb[:])
                        nc.sync.dma_start(dbg["d_rs"][:], rs[:])
                        nc.sync.dma_start(dbg["d_rbs"][:], rbs[:])
                    nc.vector.tensor_mul(yTf[0:64, r, tsl], y_ps[0:64, 0, :],
                                         rbs[:, 0, :])
                    nc.vector.tensor_mul(yTf[64:128, r, tsl], y_ps[0:64, 1, :],
                                         rbs[:, 1, :])

                # ---- stage C for this half (PE overlaps next half's B)
                for ct in range(8):
                    o_ps = pB_.tile([128, 512], f32, name="o_ps", tag="ops")
                    for kr in range(4):
                        nc.tensor.matmul(o_ps[:], wo_sb[:, kr, ts(ct, 128)],
                                         yTf[:, kr, tsl], start=(kr == 0),
                                         stop=(kr == 3))
                    o_sb = stB.tile([128, 512], bf16, name="o_sb", tag="osb",
                                    bufs=3)
                    if ccount % 2 == 0:
                        nc.scalar.activation(o_sb[:], o_ps[:], Copy)
                    else:
                        nc.vector.tensor_copy(o_sb[:], o_ps[:])
                    qdma = nc.sync if ccount % 2 == 0 else nc.scalar
                    qdma.dma_start(outT[ts(ct, 128), tsl], o_sb[:])
                    ccount += 1

        if debug:
            nc.sync.dma_start(dbg["d_qTf"][:], qTf[:])
            nc.sync.dma_start(dbg["d_kTf"][:], kTf[:])
            nc.sync.dma_start(dbg["d_v"][:], v_sb[:])
            nc.sync.dma_start(dbg["d_yTf"][:], yTf[:])

    nc.compile()
    return nc


def _prep_core_inputs(x, ve3, cosT, sinT, Wq, Wk, Wv, Wo, Wg, consts, b, s):
    """Host-side arrangement of one core's DRAM inputs (bf16)."""
    import ml_dtypes
    bf = ml_dtypes.bfloat16
    g0, g1 = 2 * s, 2 * s + 1

    # xtn[p, kc, j, tt] = x[b, 128j+tt, 128kc+p]
    xtn = np.ascontiguousarray(
        x[b].reshape(8, 128, 8, 128).transpose(3, 2, 0, 1)).astype(bf)

    Wq4 = Wq.reshape(HKV, REP, D, C)
    # wq_cols[c, r*128+gg*64+d] = Wq4[2s+gg, r, d, c]
    wq_cols = np.concatenate(
        [Wq4[g, r].T for r in range(REP) for g in (g0, g1)],
        axis=1)                                            # (C, 512)
    wqn = np.ascontiguousarray(
        wq_cols.reshape(8, 128, 512).transpose(1, 0, 2)).astype(bf)

    Wk3 = Wk.reshape(HKV, D, C)
    wk_cols = np.concatenate([Wk3[g0].T, Wk3[g1].T], axis=1)  # (C, 128)
    wkn = np.ascontiguousarray(
        wk_cols.reshape(8, 128, 128).transpose(1, 0, 2)).astype(bf)
    Wv3 = Wv.reshape(HKV, D, C)
    wv_cols = np.concatenate([Wv3[g0].T, Wv3[g1].T], axis=1)
    wvn = np.ascontiguousarray(
        wv_cols.reshape(8, 128, 128).transpose(1, 0, 2)).astype(bf)

    Wo4 = Wo.reshape(C, HKV, REP, D)
    # won[gg*64+d, r, c] = Wo4[c, 2s+gg, r, d]
    won = np.ascontiguousarray(
        np.stack([np.concatenate([Wo4[:, g0, r, :].T, Wo4[:, g1, r, :].T],
                                 axis=0) for r in range(REP)],
                 axis=1)).astype(bf)                       # (128, 4, C)

    wgn = np.zeros((16, 2), dtype=np.float32)
    wgn[0:GATE_CH, 0] = Wg[g0]
    wgn[0:GATE_CH, 1] = Wg[g1]
    wgn = wgn.astype(bf)

    ve4 = ve3[b].reshape(T, HKV, D)
    ve3n = np.ascontiguousarray(
        np.stack([ve4[:, g0, :], ve4[:, g1, :]],
                 axis=1).reshape(8, 128, 2, 64).transpose(1, 0, 2, 3)).astype(bf)

    d = dict(xtn=xtn, wqn=wqn, wkn=wkn, wvn=wvn, won=won, wgn=wgn, ve3n=ve3n)
    d.update(consts)
    return d


def _const_inputs(cosT, sinT):
    import ml_dtypes
    bf = ml_dtypes.bfloat16
    # cpat[t, d] = cos[t, d % 32]; spat[t, d] = -sin[t,d] (d<32) else sin[t,d-32]
    cfull = np.concatenate([cosT, cosT], axis=1)           # (T, 64)
    sfull = np.concatenate([-sinT, sinT], axis=1)          # (T, 64)
    cpat = np.ascontiguousarray(
        cfull.reshape(8, 128, 64).transpose(1, 0, 2)).astype(bf)
    spat = np.ascontiguousarray(
        sfull.reshape(8, 128, 64).transpose(1, 0, 2)).astype(bf)
    idx = np.arange(128)
    mlo = (idx[None, :] >= idx[:, None]).astype(bf)        # keep col >= row
    mhi = (idx[None, :] <= idx[:, None]).astype(bf)        # keep col <= row
    return dict(cpat=cpat, spat=spat, mlo=mlo, mhi=mhi)


def kernel(x, ve, cos, sin, Wq, Wk, Wv, Wo, Wg, window_size):
    from concourse.bass_utils import run_bass_kernel_spmd

    assert int(window_size) == WINDOW
    x = np.asarray(x, dtype=np.float32)
    ve3 = 3.0 * np.asarray(ve, dtype=np.float32)
    Wq = np.asarray(Wq, dtype=np.float32)
    Wk = np.asarray(Wk, dtype=np.float32)
    Wv = np.asarray(Wv, dtype=np.float32)
    Wo = np.asarray(Wo, dtype=np.float32)
    Wg = np.asarray(Wg, dtype=np.float32)
    cosT = np.asarray(cos, dtype=np.float32).reshape(T, D // 2)
    sinT = np.asarray(sin, dtype=np.float32).reshape(T, D // 2)
    consts = _const_inputs(cosT, sinT)

    if "nc" not in _CACHE:
        _CACHE["nc"] = _build_program()
    nc = _CACHE["nc"]

    in_maps = []
    for core in range(NCORES):
        b, s = core // 2, core % 2
        in_maps.append(_prep_core_inputs(x, ve3, cosT, sinT,
                                         Wq, Wk, Wv, Wo, Wg, consts, b, s))

    res = run_bass_kernel_spmd(nc, in_maps, core_ids=list(range(NCORES)))
    out = np.empty((B, T, C), dtype=np.float32)
    for b in range(B):
        acc = (res.results[2 * b]["out_t"].astype(np.float32) +
               res.results[2 * b + 1]["out_t"].astype(np.float32))
        out[b] = acc.T
    return out


# revision 49
# speedup vs baseline: 1.0826x; 1.0826x over previous
"""Sliding-window GQA causal self-attention for Trainium2, 8 NeuronCores.

Sharding: 8 cores = 4 batches x 2 head-shards. Each core handles one batch
and 2 of the 4 KV groups (8 of 16 Q heads). Core computes a full [C, T]
partial of the output projection; host sums the two shards per batch.

Design (v2, natural-layout stage A, bf16 operands):
  Projections run "natural" (tokens on partitions): per 128-token block j,
  q_ps[t, 512], k/v/gate packed in one PSUM tile. RoPE becomes free-dim
  32-shifts (no partition swaps); rms uses rotation invariance (computed
  from pre-rope PSUM via square+reduce); rstd_k folds into the PSUM->SBUF
  copy (Act copy with per-partition scale), rstd_q (with the 1.2*1.2/8
  score scale) folds into the per-head Act copies. q/k transpose via PE
  into bf16 PSUM, Pool copies them out.

  Attention: per (h-half, r): scores^T via bf16 QK matmuls, Exp on Act,
  band-edge masks via DVE multiplies with precomputed triangle masks.
  V carries a ones-column so softmax denominators fall out of the PV
  matmul (row 64 of y_ps). Normalize: DVE reciprocal -> rank-1 broadcast
  matmul -> DVE/Pool multiplies into yTf (bf16). Output projection per
  h-half interleaves with the other half's attention on the PE.
"""
import numpy as np

B, T, C = 4, 1024, 1024
H, HKV, D = 16, 4, 64
REP = H // HKV
WINDOW = 256
GATE_CH = 12
NCORES = 8
EPS = float(np.finfo(np.float32).eps)
QK_SCALE = 1.2 * 1.2 / 8.0  # two rms gains (1.2 each) * 1/sqrt(D)

_CACHE = {}


def _build_program(debug=False, reps=1):
    from contextlib import ExitStack
    import concourse.bass as bass
    import concourse.tile as tile
    from concourse import bacc, mybir
    from concourse.masks import make_identity

    f32 = mybir.dt.float32
    f32r = mybir.dt.float32r
    bf16 = mybir.dt.bfloat16
    ts = bass.ts

    nc = bacc.Bacc("TRN2", target_bir_lowering=False, debug=False,
                   enable_asserts=True, num_devices=NCORES)

    def din(name, shape, dt=bf16):
        return nc.dram_tensor(name, shape, dt, kind="ExternalInput").ap()

    # host-prearranged layouts (see _prep_core_inputs)
    xtn = din("xtn", [128, 8, 8, 128])    # [c%128, c//128, j, t%128] = x^T
    wqn = din("wqn", [128, 8, 512])       # [c%128, c//128, r*128+gg*64+d]
    wkn = din("wkn", [128, 8, 128])       # [c%128, c//128, gg*64+d]
    wvn = din("wvn", [128, 8, 128])
    won = din("won", [128, 4, 1024])      # [gg*64+d, r, c]
    wgn = din("wgn", [16, 2])             # zero-padded 12->16 gate rows
    ve3n = din("ve3n", [128, 8, 2, 64])   # [t%128, j, gg, d] = 3*ve
    cpat = din("cpat", [128, 8, 64])      # cos[t, d%32]
    spat = din("spat", [128, 8, 64])      # -sin[t,d] / +sin[t,d-32] halves
    mlo = din("mlo", [128, 128])          # causal edge: 1 if col >= row
    mhi = din("mhi", [128, 128])          # window edge: 1 if col <= row
    outT = nc.dram_tensor("out_t", [C, T], bf16, kind="ExternalOutput").ap()
    dbg = {}
    if debug:
        for nm, shp in [("d_qTf", [128, 4, T]), ("d_kTf", [128, T]),
                        ("d_v", [128, 8, 2, 65]), ("d_yTf", [128, 4, T]),
                        ("d_ks", [128, 8, 2, 64]), ("d_kr", [128, 8, 2, 64]),
                        ("d_rsk", [128, 8, 2]), ("d_msqk", [128, 8, 2]),
                        ("d_kraw", [128, 8, 2, 64]), ("d_k2", [128, 8, 2, 64]),
                        ("d_rs", [1, 2, 512]), ("d_rbs", [64, 2, 512]),
                        ("d_sums", [1, 2, 512])]:
            dt = f32 if nm in ("d_rsk", "d_msqk", "d_rs", "d_rbs",
                               "d_sums") else bf16
            dbg[nm] = nc.dram_tensor(nm, shp, dt, kind="ExternalOutput").ap()

    Exp = mybir.ActivationFunctionType.Exp
    Square = mybir.ActivationFunctionType.Square
    Sqrt = mybir.ActivationFunctionType.Sqrt
    Copy = mybir.ActivationFunctionType.Copy
    add_op = mybir.AluOpType.add
    mult_op = mybir.AluOpType.mult

    with tile.TileContext(nc) as tc:
     for _rep in range(reps):
      with ExitStack() as ctx:
        sing = ctx.enter_context(tc.tile_pool(name="sing", bufs=1))

        # ---------- persistent tiles + loads ----------
        # SP queue: xt0/wk/wv first so tblock-0 k/v matmuls start ASAP;
        # Act queue takes the rope/gate tables + wo (needed later).
        xt_sb = [sing.tile([128, 8, 128], bf16, name=f"xt{j}") for j in range(8)]
        nc.sync.dma_start(xt_sb[0][:], xtn[:, :, 0, :])
        wk_sb = sing.tile([128, 8, 128], bf16, name="wk_sb")
        nc.sync.dma_start(wk_sb[:], wkn[:])
        wv_sb = sing.tile([128, 8, 128], bf16, name="wv_sb")
        nc.sync.dma_start(wv_sb[:], wvn[:])
        wg_sb = sing.tile([16, 2], bf16, name="wg_sb")
        nc.sync.dma_start(wg_sb[:], wgn[:])
        nc.sync.dma_start(xt_sb[1][:], xtn[:, :, 1, :])
        wq_sb = sing.tile([128, 8, 512], bf16, name="wq_sb")
        nc.sync.dma_start(wq_sb[:, 0:4, :], wqn[:, 0:4, :])
        nc.sync.dma_start(xt_sb[2][:], xtn[:, :, 2, :])
        nc.sync.dma_start(wq_sb[:, 4:8, :], wqn[:, 4:8, :])
        for j in range(3, 8):
            nc.sync.dma_start(xt_sb[j][:], xtn[:, :, j, :])
        cpat_sb = sing.tile([128, 8, 64], bf16, name="cpat_sb")
        nc.scalar.dma_start(cpat_sb[:], cpat[:])
        spat_sb = sing.tile([128, 8, 64], bf16, name="spat_sb")
        nc.scalar.dma_start(spat_sb[:], spat[:])
        ve_sb = sing.tile([128, 8, 2, 64], bf16, name="ve_sb")
        nc.scalar.dma_start(ve_sb[:], ve3n[:])
        mlo_sb = sing.tile([128, 128], bf16, name="mlo_sb")
        nc.scalar.dma_start(mlo_sb[:], mlo[:])
        mhi_sb = sing.tile([128, 128], bf16, name="mhi_sb")
        nc.scalar.dma_start(mhi_sb[:], mhi[:])
        # wo only needed by stage C
        wo_sb = sing.tile([128, 4, 1024], bf16, name="wo_sb")
        nc.scalar.dma_start(wo_sb[:], won[:])

        ident = sing.tile([128, 128], bf16, name="ident")
        make_identity(nc, ident[:])

        epsk_sb = sing.tile([128, 1], f32, name="epsk_sb")
        nc.vector.memset(epsk_sb[:], EPS)
        epsq_sb = sing.tile([128, 1], f32, name="epsq_sb")
        nc.vector.memset(epsq_sb[:], EPS / (QK_SCALE * QK_SCALE))
        qTf = sing.tile([128, 4, T], bf16, name="qTf")   # [(gg,d), r, t]
        kTf = sing.tile([128, T], bf16, name="kTf")      # [(gg,d), t]
        v_sb = sing.tile([128, 8, 2, 65], bf16, name="v_sb")  # [t%128, j, gg, d|1]
        yTf = sing.tile([128, 4, T], bf16, name="yTf")
        nc.vector.memset(v_sb[:, :, :, 64:65], 1.0)

        # ================= Stage A: projections / rope / rms / gate =========
        # Software-pipelined: transposes for block j-1 are emitted after
        # block j's matmuls so the PE never stalls on the vector chain.
        with tc.tile_pool(name="stA", bufs=2) as stA, \
             tc.tile_pool(name="pA_", bufs=2, space="PSUM") as pA_:
            pending = []

            def emit_transpose(j, qr, kr):
                tp = pA_.tile([128, 640], bf16, name="tp", tag="tp")
                for r in range(4):
                    nc.tensor.transpose(
                        tp[:, ts(r, 128)],
                        qr[:, 2 * r:2 * r + 2, :].rearrange("p g d -> p (g d)"),
                        ident[:])
                nc.tensor.transpose(
                    tp[:, 512:640], kr[:].rearrange("p g d -> p (g d)"),
                    ident[:])
                nc.scalar.activation(
                    qTf[:, :, ts(j, 128)],
                    tp[:, 0:512].rearrange("p (r t) -> p r t", r=4), Copy)
                nc.vector.tensor_copy(kTf[:, ts(j, 128)], tp[:, 512:640])

            for j in range(8):
                xt = xt_sb[j]
                # ---- k / v / gate packed in one PSUM bank
                # k, v, gate groups must be sequential: interleaving two
                # accumulation groups in one PSUM bank loses contributions.
                kvg = pA_.tile([128, 258], f32, name="kvg", tag="kvg")
                for kc in range(8):
                    nc.tensor.matmul(kvg[:, 0:128], xt[:, kc, :],
                                     wk_sb[:, kc, :], start=(kc == 0),
                                     stop=(kc == 7), skip_group_check=True)
                for kc in range(8):
                    nc.tensor.matmul(kvg[:, 128:256], xt[:, kc, :],
                                     wv_sb[:, kc, :], start=(kc == 0),
                                     stop=(kc == 7), skip_group_check=True)
                nc.tensor.matmul(kvg[:, 256:258], xt[0:16, 0, :], wg_sb[:],
                                 start=True, stop=True, skip_group_check=True)
                # ---- q projection
                q_ps = pA_.tile([128, 8, 64], f32, name="q_ps", tag="qps")
                for kc in range(8):
                    nc.tensor.matmul(q_ps[:].rearrange("p h d -> p (h d)"),
                                     xt[:, kc, :], wq_sb[:, kc, :],
                                     start=(kc == 0), stop=(kc == 7))
                # previous block's transposes ride behind this block's matmuls
                if pending:
                    emit_transpose(*pending.pop())

                kk = kvg[:, 0:128].rearrange("p (g d) -> p g d", g=2)
                # ---- k rms (pre-rope; rotation preserves norms).
                # Squares on Act (PSUM ops may read only one PSUM input).
                k2 = stA.tile([128, 2, 64], bf16, name="k2", tag="k2")
                nc.scalar.activation(k2[:], kk, Square)
                msqk = stA.tile([128, 2], f32, name="msqk", tag="msqk")
                nc.vector.tensor_reduce(msqk[:], k2[:], mybir.AxisListType.X,
                                        add_op)
                sqk = stA.tile([128, 2], f32, name="sqk", tag="sqk")
                nc.scalar.activation(sqk[:], msqk[:], Sqrt, scale=1.0 / D,
                                     bias=epsk_sb[:])
                rsk = stA.tile([128, 2], f32, name="rsk", tag="rsk")
                nc.vector.reciprocal(rsk[:], sqk[:])
                # ---- k: PSUM->SBUF with rstd_k folded into the copy
                ks = stA.tile([128, 2, 64], bf16, name="ks", tag="ks")
                for gg in range(2):
                    nc.vector.tensor_scalar_mul(ks[:, gg, :], kk[:, gg, :],
                                                rsk[:, gg:gg + 1])
                # ---- k rope (Pool): kr = ks*cpat + shift32(ks*spat)
                spat_j2 = spat_sb[:, j:j + 1, :].broadcast_to([128, 2, 64])
                cpat_j2 = cpat_sb[:, j:j + 1, :].broadcast_to([128, 2, 64])
                ku = stA.tile([128, 2, 64], bf16, name="ku", tag="ku")
                nc.gpsimd.tensor_mul(ku[:], ks[:], spat_j2)
                kr = stA.tile([128, 2, 64], bf16, name="kr", tag="kr")
                nc.gpsimd.tensor_mul(kr[:], ks[:], cpat_j2)
                nc.vector.tensor_add(kr[:, :, 0:32], kr[:, :, 0:32],
                                     ku[:, :, 32:64])
                nc.vector.tensor_add(kr[:, :, 32:64], kr[:, :, 32:64],
                                     ku[:, :, 0:32])
                if debug:
                    nc.sync.dma_start(dbg["d_ks"][:, j, :, :], ks[:])
                    nc.sync.dma_start(dbg["d_kr"][:, j, :, :], kr[:])
                    nc.sync.dma_start(dbg["d_rsk"][:, j, :], rsk[:])
                    nc.sync.dma_start(dbg["d_msqk"][:, j, :], msqk[:])
                    kraw = stA.tile([128, 2, 64], bf16, name="kraw", tag="kraw")
                    nc.vector.tensor_copy(kraw[:], kk)
                    nc.sync.dma_start(dbg["d_kraw"][:, j, :, :], kraw[:])
                    nc.sync.dma_start(dbg["d_k2"][:, j, :, :], k2[:])

                # ---- v + gate*ve (gate = 3*sigmoid, the 3 is folded in ve3).
                # g = x[:, :12] @ Wg^T is tiny (|g| < ~0.5 since Wg ~ 0.02),
                # so sigmoid(g) ~= 0.5 + g*(0.25 - g^2/48), err < 2e-4.
                gg2 = stA.tile([128, 2], f32, name="gg2", tag="gg2")
                nc.scalar.activation(gg2[:], kvg[:, 256:258], Square)
                nc.vector.tensor_scalar(gg2[:], gg2[:], -1.0 / 48.0, 0.25,
                                        mult_op, add_op)
                sg = stA.tile([128, 2], f32, name="sg", tag="sg")
                nc.vector.tensor_mul(sg[:], gg2[:], kvg[:, 256:258])
                nc.vector.tensor_scalar_add(sg[:], sg[:], 0.5)
                gve = stA.tile([128, 2, 64], bf16, name="gve", tag="gve")
                for gg in range(2):
                    nc.gpsimd.tensor_scalar_mul(gve[:, gg, :],
                                                ve_sb[:, j, gg, :],
                                                sg[:, gg:gg + 1])
                nc.vector.tensor_add(
                    v_sb[:, j, :, 0:64],
                    kvg[:, 128:256].rearrange("p (g d) -> p g d", g=2), gve[:])

                # ---- q rms (pre-rope), rstd_q folded with QK_SCALE
                q2 = stA.tile([128, 8, 64], bf16, name="q2", tag="q2")
                nc.scalar.activation(q2[:], q_ps[:], Square)
                msqq = stA.tile([128, 8], f32, name="msqq", tag="msqq")
                nc.vector.tensor_reduce(msqq[:], q2[:], mybir.AxisListType.X,
                                        add_op)
                sqq = stA.tile([128, 8], f32, name="sqq", tag="sqq")
                nc.scalar.activation(sqq[:], msqq[:], Sqrt,
                                     scale=1.0 / (D * QK_SCALE * QK_SCALE),
                                     bias=epsq_sb[:])
                rsq = stA.tile([128, 8], f32, name="rsq", tag="rsq")
                nc.vector.reciprocal(rsq[:], sqq[:])
                # ---- q: PSUM->SBUF bf16 with per-head rstd_q scale
                qs = stA.tile([128, 8, 64], bf16, name="qs", tag="qs")
                for h8 in range(8):
                    nc.vector.tensor_scalar_mul(qs[:, h8, :], q_ps[:, h8, :],
                                                rsq[:, h8:h8 + 1])
                # ---- q rope (muls on Pool, shift-adds on DVE)
                spat_j8 = spat_sb[:, j:j + 1, :].broadcast_to([128, 8, 64])
                cpat_j8 = cpat_sb[:, j:j + 1, :].broadcast_to([128, 8, 64])
                qu = stA.tile([128, 8, 64], bf16, name="qu", tag="qu")
                nc.gpsimd.tensor_mul(qu[:], qs[:], spat_j8)
                qr = stA.tile([128, 8, 64], bf16, name="qr", tag="qr")
                nc.gpsimd.tensor_mul(qr[:], qs[:], cpat_j8)
                nc.vector.tensor_add(qr[:, :, 0:32], qr[:, :, 0:32],
                                     qu[:, :, 32:64])
                nc.vector.tensor_add(qr[:, :, 32:64], qr[:, :, 32:64],
                                     qu[:, :, 0:32])

                pending.append((j, qr, kr))
            emit_transpose(*pending.pop())

        # ================= Stage B + C: attention + out projection ==========
        with tc.tile_pool(name="stB", bufs=2) as stB, \
             tc.tile_pool(name="pB_", bufs=1, space="PSUM") as pB_:
            pkeep = {(r, j): stB.tile([128, 2, 384], bf16, name=f"pk{r}{j}",
                                      tag=f"pk{r}{j}", bufs=1)
                     for r in range(4) for j in (2, 3)}
            ccount = 0
            for h in range(2):
                tsl = slice(512 * h, 512 * h + 512)
                jlist = list(range(0, 4)) if h == 0 else list(range(2, 8))
                for r in range(4):
                    y_ps = pB_.tile([65, 2, 512], f32, name="y_ps", tag="yps",
                                    bufs=2)
                    pvq = []  # PV lags QK/exp/mask by one block

                    def emit_pv(j, p2, first, last):
                        w = min(384, T - 128 * j)
                        a = max(128 * j, 512 * h)
                        b = min(128 * j + w, 512 * h + 512)
                        n0, nn = a - 128 * j, b - a
                        for gg in range(2):
                            nc.tensor.matmul(
                                y_ps[:, gg, a - 512 * h:b - 512 * h],
                                v_sb[:, j, gg, :], p2[:, gg, n0:n0 + nn],
                                start=first, stop=last,
                                skip_group_check=True)

                    for j in jlist:
                        w = min(384, T - 128 * j)
                        if h == 1 and j in (2, 3):
                            p2 = pkeep[(r, j)]  # cached from h == 0
                        else:
                            p2 = (pkeep[(r, j)] if j in (2, 3) else
                                  stB.tile([128, 2, 384], bf16, name="p2",
                                           tag="p2", bufs=3))
                            sc2 = pB_.tile([128, 2, 512], f32, name="sc2",
                                           tag="sc", bufs=2)
                            for gg in range(2):
                                nc.tensor.matmul(
                                    sc2[:, gg, 0:w],
                                    kTf[ts(gg, 64), ts(j, 128)],
                                    qTf[ts(gg, 64), r, 128 * j:128 * j + w],
                                    start=True, stop=True)
                            nc.scalar.activation(p2[:, :, 0:w], sc2[:, :, 0:w],
                                                 Exp)
                            # band edges: causal lower, window upper triangle
                            nc.vector.tensor_mul(
                                p2[:, :, 0:128], p2[:, :, 0:128],
                                mlo_sb[:].unsqueeze(1).broadcast_to([128, 2, 128]))
                            if w > 256:
                                nc.vector.tensor_mul(
                                    p2[:, :, 256:384], p2[:, :, 256:384],
                                    mhi_sb[:].unsqueeze(1).broadcast_to([128, 2, 128]))
                        pvq.append((j, p2))
                        if len(pvq) > 1:
                            jp, pp = pvq.pop(0)
                            emit_pv(jp, pp, jp == jlist[0], False)
                    jp, pp = pvq.pop(0)
                    emit_pv(jp, pp, jp == jlist[0], True)
                    # normalize: 1/sums (row 64) -> rank-1 broadcast -> apply
                    su = stB.tile([1, 2, 512], f32, name="su", tag="su")
                    nc.scalar.activation(su[:], y_ps[64:65, :, :], Copy)
                    rs = stB.tile([1, 2, 512], f32, name="rs", tag="rs")
                    nc.vector.reciprocal_approx_fast(rs[:], su[:])
                    rbs = stB.tile([64, 2, 512], f32, name="rbs", tag="rbs")
                    nc.gpsimd.partition_broadcast(rbs[:], rs[:], channels=64)
                    if debug and h == 0 and r == 0:
                        sums_sb = stB.tile([1, 2, 512], f32, name="sums_sb",
                                           tag="sums_sb")
                        nc.vector.tensor_copy(sums_sb[:], y_ps[64:65, :, :])
                        nc.sync.dma_start(dbg["d_sums"][:], sums_sb[:])
                        nc.sync.dma_start(dbg["d_rs"][:], rs[:])
                        nc.sync.dma_start(dbg["d_rbs"][:], rbs[:])
                    nc.vector.tensor_mul(yTf[0:64, r, tsl], y_ps[0:64, 0, :],
                                         rbs[:, 0, :])
                    nc.vector.tensor_mul(yTf[64:128, r, tsl], y_ps[0:64, 1, :],
                                         rbs[:, 1, :])

                # ---- stage C for this half (PE overlaps next half's B).
                # o_ps shares the "sc" buffer rotation to stay within 8 banks.
                for ct in range(8):
                    o_full = pB_.tile([128, 2, 512], f32, name="sc2", tag="sc",
                                      bufs=2)
                    o_ps = o_full[:, 0, :]
                    for kr in range(4):
                        nc.tensor.matmul(o_ps, wo_sb[:, kr, ts(ct, 128)],
                                         yTf[:, kr, tsl], start=(kr == 0),
                                         stop=(kr == 3))
                    o_sb = stB.tile([128, 512], bf16, name="o_sb", tag="osb",
                                    bufs=3)
                    if ccount % 2 == 0:
                        nc.scalar.activation(o_sb[:], o_ps, Copy)
                    else:
                        nc.vector.tensor_copy(o_sb[:], o_ps)
                    qdma = nc.sync if ccount % 2 == 0 else nc.scalar
                    qdma.dma_start(outT[ts(ct, 128), tsl], o_sb[:])
                    ccount += 1

        if debug:
            nc.sync.dma_start(dbg["d_qTf"][:], qTf[:])
            nc.sync.dma_start(dbg["d_kTf"][:], kTf[:])
            nc.sync.dma_start(dbg["d_v"][:], v_sb[:])
            nc.sync.dma_start(dbg["d_yTf"][:], yTf[:])

    nc.compile()
    return nc


def _prep_core_inputs(x, ve3, cosT, sinT, Wq, Wk, Wv, Wo, Wg, consts, b, s):
    """Host-side arrangement of one core's DRAM inputs (bf16)."""
    import ml_dtypes
    bf = ml_dtypes.bfloat16
    g0, g1 = 2 * s, 2 * s + 1

    # xtn[p, kc, j, tt] = x[b, 128j+tt, 128kc+p]
    xtn = np.ascontiguousarray(
        x[b].reshape(8, 128, 8, 128).transpose(3, 2, 0, 1)).astype(bf)

    Wq4 = Wq.reshape(HKV, REP, D, C)
    # wq_cols[c, r*128+gg*64+d] = Wq4[2s+gg, r, d, c]
    wq_cols = np.concatenate(
        [Wq4[g, r].T for r in range(REP) for g in (g0, g1)],
        axis=1)                                            # (C, 512)
    wqn = np.ascontiguousarray(
        wq_cols.reshape(8, 128, 512).transpose(1, 0, 2)).astype(bf)

    Wk3 = Wk.reshape(HKV, D, C)
    wk_cols = np.concatenate([Wk3[g0].T, Wk3[g1].T], axis=1)  # (C, 128)
    wkn = np.ascontiguousarray(
        wk_cols.reshape(8, 128, 128).transpose(1, 0, 2)).astype(bf)
    Wv3 = Wv.reshape(HKV, D, C)
    wv_cols = np.concatenate([Wv3[g0].T, Wv3[g1].T], axis=1)
    wvn = np.ascontiguousarray(
        wv_cols.reshape(8, 128, 128).transpose(1, 0, 2)).astype(bf)

    Wo4 = Wo.reshape(C, HKV, REP, D)
    # won[gg*64+d, r, c] = Wo4[c, 2s+gg, r, d]
    won = np.ascontiguousarray(
        np.stack([np.concatenate([Wo4[:, g0, r, :].T, Wo4[:, g1, r, :].T],
                                 axis=0) for r in range(REP)],
                 axis=1)).astype(bf)                       # (128, 4, C)

    wgn = np.zeros((16, 2), dtype=np.float32)
    wgn[0:GATE_CH, 0] = Wg[g0]
    wgn[0:GATE_CH, 1] = Wg[g1]
    wgn = wgn.astype(bf)

    ve4 = ve3[b].reshape(T, HKV, D)
    ve3n = np.ascontiguousarray(
        np.stack([ve4[:, g0, :], ve4[:, g1, :]],
                 axis=1).reshape(8, 128, 2, 64).transpose(1, 0, 2, 3)).astype(bf)

    d = dict(xtn=xtn, wqn=wqn, wkn=wkn, wvn=wvn, won=won, wgn=wgn, ve3n=ve3n)
    d.update(consts)
    return d


def _const_inputs(cosT, sinT):
    import ml_dtypes
    bf = ml_dtypes.bfloat16
    # cpat[t, d] = cos[t, d % 32]; spat[t, d] = -sin[t,d] (d<32) else sin[t,d-32]
    cfull = np.concatenate([cosT, cosT], axis=1)           # (T, 64)
    sfull = np.concatenate([-sinT, sinT], axis=1)          # (T, 64)
    cpat = np.ascontiguousarray(
        cfull.reshape(8, 128, 64).transpose(1, 0, 2)).astype(bf)
    spat = np.ascontiguousarray(
        sfull.reshape(8, 128, 64).transpose(1, 0, 2)).astype(bf)
    idx = np.arange(128)
    mlo = (idx[None, :] >= idx[:, None]).astype(bf)        # keep col >= row
    mhi = (idx[None, :] <= idx[:, None]).astype(bf)        # keep col <= row
    return dict(cpat=cpat, spat=spat, mlo=mlo, mhi=mhi)


def kernel(x, ve, cos, sin, Wq, Wk, Wv, Wo, Wg, window_size):
    from concourse.bass_utils import run_bass_kernel_spmd

    assert int(window_size) == WINDOW
    x = np.asarray(x, dtype=np.float32)
    ve3 = 3.0 * np.asarray(ve, dtype=np.float32)
    Wq = np.asarray(Wq, dtype=np.float32)
    Wk = np.asarray(Wk, dtype=np.float32)
    Wv = np.asarray(Wv, dtype=np.float32)
    Wo = np.asarray(Wo, dtype=np.float32)
    Wg = np.asarray(Wg, dtype=np.float32)
    cosT = np.asarray(cos, dtype=np.float32).reshape(T, D // 2)
    sinT = np.asarray(sin, dtype=np.float32).reshape(T, D // 2)
    consts = _const_inputs(cosT, sinT)

    if "nc" not in _CACHE:
        _CACHE["nc"] = _build_program()
    nc = _CACHE["nc"]

    in_maps = []
    for core in range(NCORES):
        b, s = core // 2, core % 2
        in_maps.append(_prep_core_inputs(x, ve3, cosT, sinT,
                                         Wq, Wk, Wv, Wo, Wg, consts, b, s))

    res = run_bass_kernel_spmd(nc, in_maps, core_ids=list(range(NCORES)))
    out = np.empty((B, T, C), dtype=np.float32)
    for b in range(B):
        acc = (res.results[2 * b]["out_t"].astype(np.float32) +
               res.results[2 * b + 1]["out_t"].astype(np.float32))
        out[b] = acc.T
    return out


# revision 50
# speedup vs baseline: 1.1065x; 1.0220x over previous
"""Sliding-window GQA causal self-attention for Trainium2, 8 NeuronCores.

Sharding: 8 cores = 4 batches x 2 head-shards. Each core handles one batch
and 2 of the 4 KV groups (8 of 16 Q heads). Core computes a full [C, T]
partial of the output projection; host sums the two shards per batch.

Design (v2, natural-layout stage A, bf16 operands):
  Projections run "natural" (tokens on partitions): per 128-token block j,
  q_ps[t, 512], k/v/gate packed in one PSUM tile. RoPE becomes free-dim
  32-shifts (no partition swaps); rms uses rotation invariance (computed
  from pre-rope PSUM via square+reduce); rstd_k folds into the PSUM->SBUF
  copy (Act copy with per-partition scale), rstd_q (with the 1.2*1.2/8
  score scale) folds into the per-head Act copies. q/k transpose via PE
  into bf16 PSUM, Pool copies them out.

  Attention: per (h-half, r): scores^T via bf16 QK matmuls, Exp on Act,
  band-edge masks via DVE multiplies with precomputed triangle masks.
  V carries a ones-column so softmax denominators fall out of the PV
  matmul (row 64 of y_ps). Normalize: DVE reciprocal -> rank-1 broadcast
  matmul -> DVE/Pool multiplies into yTf (bf16). Output projection per
  h-half interleaves with the other half's attention on the PE.
"""
import numpy as np

B, T, C = 4, 1024, 1024
H, HKV, D = 16, 4, 64
REP = H // HKV
WINDOW = 256
GATE_CH = 12
NCORES = 8
EPS = float(np.finfo(np.float32).eps)
QK_SCALE = 1.2 * 1.2 / 8.0  # two rms gains (1.2 each) * 1/sqrt(D)

_CACHE = {}


def _build_program(debug=False, reps=1):
    from contextlib import ExitStack
    import concourse.bass as bass
    import concourse.tile as tile
    from concourse import bacc, mybir
    from concourse.masks import make_identity

    f32 = mybir.dt.float32
    f32r = mybir.dt.float32r
    bf16 = mybir.dt.bfloat16
    ts = bass.ts

    nc = bacc.Bacc("TRN2", target_bir_lowering=False, debug=False,
                   enable_asserts=True, num_devices=NCORES)

    def din(name, shape, dt=bf16):
        return nc.dram_tensor(name, shape, dt, kind="ExternalInput").ap()

    # host-prearranged layouts (see _prep_core_inputs)
    xtn = din("xtn", [128, 8, 8, 128])    # [c%128, c//128, j, t%128] = x^T
    wqn = din("wqn", [128, 8, 512])       # [c%128, c//128, r*128+gg*64+d]
    wkn = din("wkn", [128, 8, 128])       # [c%128, c//128, gg*64+d]
    wvn = din("wvn", [128, 8, 128])
    won = din("won", [128, 4, 1024])      # [gg*64+d, r, c]
    wgn = din("wgn", [16, 2])             # zero-padded 12->16 gate rows
    ve3n = din("ve3n", [128, 8, 2, 64])   # [t%128, j, gg, d] = 3*ve
    cpat = din("cpat", [128, 8, 64])      # cos[t, d%32]
    spat = din("spat", [128, 8, 64])      # -sin[t,d] / +sin[t,d-32] halves
    mlo = din("mlo", [128, 128])          # causal edge: 1 if col >= row
    mhi = din("mhi", [128, 128])          # window edge: 1 if col <= row
    outT = nc.dram_tensor("out_t", [C, T], bf16, kind="ExternalOutput").ap()
    dbg = {}
    if debug:
        for nm, shp in [("d_qTf", [128, 4, T]), ("d_kTf", [128, T]),
                        ("d_v", [128, 8, 2, 65]), ("d_yTf", [128, 4, T]),
                        ("d_ks", [128, 8, 2, 64]), ("d_kr", [128, 8, 2, 64]),
                        ("d_rsk", [128, 8, 2]), ("d_msqk", [128, 8, 2]),
                        ("d_kraw", [128, 8, 2, 64]), ("d_k2", [128, 8, 2, 64]),
                        ("d_rs", [1, 2, 512]), ("d_rbs", [64, 2, 512]),
                        ("d_sums", [1, 2, 512])]:
            dt = f32 if nm in ("d_rsk", "d_msqk", "d_rs", "d_rbs",
                               "d_sums") else bf16
            dbg[nm] = nc.dram_tensor(nm, shp, dt, kind="ExternalOutput").ap()

    Exp = mybir.ActivationFunctionType.Exp
    Square = mybir.ActivationFunctionType.Square
    Sqrt = mybir.ActivationFunctionType.Sqrt
    Copy = mybir.ActivationFunctionType.Copy
    add_op = mybir.AluOpType.add
    mult_op = mybir.AluOpType.mult

    with tile.TileContext(nc) as tc:
     for _rep in range(reps):
      with ExitStack() as ctx:
        sing = ctx.enter_context(tc.tile_pool(name="sing", bufs=1))

        # ---------- persistent tiles + loads ----------
        # SP queue: xt0/wk/wv first so tblock-0 k/v matmuls start ASAP;
        # Act queue takes the rope/gate tables + wo (needed later).
        xt_sb = [sing.tile([128, 8, 128], bf16, name=f"xt{j}") for j in range(8)]
        nc.sync.dma_start(xt_sb[0][:], xtn[:, :, 0, :])
        wk_sb = sing.tile([128, 8, 128], bf16, name="wk_sb")
        nc.sync.dma_start(wk_sb[:], wkn[:])
        wv_sb = sing.tile([128, 8, 128], bf16, name="wv_sb")
        nc.sync.dma_start(wv_sb[:], wvn[:])
        wg_sb = sing.tile([16, 2], bf16, name="wg_sb")
        nc.sync.dma_start(wg_sb[:], wgn[:])
        nc.sync.dma_start(xt_sb[1][:], xtn[:, :, 1, :])
        wq_sb = sing.tile([128, 8, 512], bf16, name="wq_sb")
        nc.sync.dma_start(wq_sb[:, 0:4, :], wqn[:, 0:4, :])
        nc.sync.dma_start(xt_sb[2][:], xtn[:, :, 2, :])
        nc.sync.dma_start(wq_sb[:, 4:8, :], wqn[:, 4:8, :])
        for j in range(3, 8):
            nc.sync.dma_start(xt_sb[j][:], xtn[:, :, j, :])
        cpat_sb = sing.tile([128, 8, 64], bf16, name="cpat_sb")
        nc.scalar.dma_start(cpat_sb[:], cpat[:])
        spat_sb = sing.tile([128, 8, 64], bf16, name="spat_sb")
        nc.scalar.dma_start(spat_sb[:], spat[:])
        ve_sb = sing.tile([128, 8, 2, 64], bf16, name="ve_sb")
        nc.scalar.dma_start(ve_sb[:], ve3n[:])
        mlo_sb = sing.tile([128, 128], bf16, name="mlo_sb")
        nc.scalar.dma_start(mlo_sb[:], mlo[:])
        mhi_sb = sing.tile([128, 128], bf16, name="mhi_sb")
        nc.scalar.dma_start(mhi_sb[:], mhi[:])
        # wo only needed by stage C
        wo_sb = sing.tile([128, 4, 1024], bf16, name="wo_sb")
        nc.scalar.dma_start(wo_sb[:], won[:])

        ident = sing.tile([128, 128], bf16, name="ident")
        make_identity(nc, ident[:])

        epsk_sb = sing.tile([128, 1], f32, name="epsk_sb")
        nc.vector.memset(epsk_sb[:], EPS)
        epsq_sb = sing.tile([128, 1], f32, name="epsq_sb")
        nc.vector.memset(epsq_sb[:], EPS / (QK_SCALE * QK_SCALE))
        qTf = sing.tile([128, 4, T], bf16, name="qTf")   # [(gg,d), r, t]
        kTf = sing.tile([128, T], bf16, name="kTf")      # [(gg,d), t]
        v_sb = sing.tile([128, 8, 2, 65], bf16, name="v_sb")  # [t%128, j, gg, d|1]
        yTf = sing.tile([128, 4, T], bf16, name="yTf")
        nc.vector.memset(v_sb[:, :, :, 64:65], 1.0)

        # ================= Stage A: projections / rope / rms / gate =========
        # Software-pipelined: transposes for block j-1 are emitted after
        # block j's matmuls so the PE never stalls on the vector chain.
        with tc.tile_pool(name="stA", bufs=2) as stA, \
             tc.tile_pool(name="pA_", bufs=2, space="PSUM") as pA_:
            pending = []

            def emit_transpose(j, qr, kr):
                tp = pA_.tile([128, 640], bf16, name="tp", tag="tp")
                for r in range(4):
                    nc.tensor.transpose(
                        tp[:, ts(r, 128)],
                        qr[:, 2 * r:2 * r + 2, :].rearrange("p g d -> p (g d)"),
                        ident[:])
                nc.tensor.transpose(
                    tp[:, 512:640], kr[:].rearrange("p g d -> p (g d)"),
                    ident[:])
                nc.scalar.activation(
                    qTf[:, :, ts(j, 128)],
                    tp[:, 0:512].rearrange("p (r t) -> p r t", r=4), Copy)
                nc.vector.tensor_copy(kTf[:, ts(j, 128)], tp[:, 512:640])

            for j in range(8):
                xt = xt_sb[j]
                # ---- k / v / gate packed in one PSUM bank
                # k, v, gate groups must be sequential: interleaving two
                # accumulation groups in one PSUM bank loses contributions.
                kvg = pA_.tile([128, 258], f32, name="kvg", tag="kvg")
                for kc in range(8):
                    nc.tensor.matmul(kvg[:, 0:128], xt[:, kc, :],
                                     wk_sb[:, kc, :], start=(kc == 0),
                                     stop=(kc == 7), skip_group_check=True)
                for kc in range(8):
                    nc.tensor.matmul(kvg[:, 128:256], xt[:, kc, :],
                                     wv_sb[:, kc, :], start=(kc == 0),
                                     stop=(kc == 7), skip_group_check=True)
                nc.tensor.matmul(kvg[:, 256:258], xt[0:16, 0, :], wg_sb[:],
                                 start=True, stop=True, skip_group_check=True)
                # ---- q projection
                q_ps = pA_.tile([128, 8, 64], f32, name="q_ps", tag="qps")
                for kc in range(8):
                    nc.tensor.matmul(q_ps[:].rearrange("p h d -> p (h d)"),
                                     xt[:, kc, :], wq_sb[:, kc, :],
                                     start=(kc == 0), stop=(kc == 7))
                # previous block's transposes ride behind this block's matmuls
                if pending:
                    emit_transpose(*pending.pop())

                kk = kvg[:, 0:128].rearrange("p (g d) -> p g d", g=2)
                # ---- k rms (pre-rope; rotation preserves norms).
                # Squares on Act (PSUM ops may read only one PSUM input).
                k2 = stA.tile([128, 2, 64], bf16, name="k2", tag="k2")
                nc.scalar.activation(k2[:], kk, Square)
                msqk = stA.tile([128, 2], f32, name="msqk", tag="msqk")
                nc.vector.tensor_reduce(msqk[:], k2[:], mybir.AxisListType.X,
                                        add_op)
                sqk = stA.tile([128, 2], f32, name="sqk", tag="sqk")
                nc.scalar.activation(sqk[:], msqk[:], Sqrt, scale=1.0 / D,
                                     bias=epsk_sb[:])
                rsk = stA.tile([128, 2], f32, name="rsk", tag="rsk")
                nc.vector.reciprocal(rsk[:], sqk[:])
                # ---- k: PSUM->SBUF with rstd_k folded into the copy
                ks = stA.tile([128, 2, 64], bf16, name="ks", tag="ks")
                for gg in range(2):
                    nc.scalar.activation(ks[:, gg, :], kk[:, gg, :], Copy,
                                         scale=rsk[:, gg:gg + 1])
                # ---- k rope (Pool): kr = ks*cpat + shift32(ks*spat)
                spat_j2 = spat_sb[:, j:j + 1, :].broadcast_to([128, 2, 64])
                cpat_j2 = cpat_sb[:, j:j + 1, :].broadcast_to([128, 2, 64])
                ku = stA.tile([128, 2, 64], bf16, name="ku", tag="ku")
                nc.gpsimd.tensor_mul(ku[:], ks[:], spat_j2)
                kr = stA.tile([128, 2, 64], bf16, name="kr", tag="kr")
                nc.gpsimd.tensor_mul(kr[:], ks[:], cpat_j2)
                nc.vector.tensor_add(kr[:, :, 0:32], kr[:, :, 0:32],
                                     ku[:, :, 32:64])
                nc.vector.tensor_add(kr[:, :, 32:64], kr[:, :, 32:64],
                                     ku[:, :, 0:32])
                if debug:
                    nc.sync.dma_start(dbg["d_ks"][:, j, :, :], ks[:])
                    nc.sync.dma_start(dbg["d_kr"][:, j, :, :], kr[:])
                    nc.sync.dma_start(dbg["d_rsk"][:, j, :], rsk[:])
                    nc.sync.dma_start(dbg["d_msqk"][:, j, :], msqk[:])
                    kraw = stA.tile([128, 2, 64], bf16, name="kraw", tag="kraw")
                    nc.vector.tensor_copy(kraw[:], kk)
                    nc.sync.dma_start(dbg["d_kraw"][:, j, :, :], kraw[:])
                    nc.sync.dma_start(dbg["d_k2"][:, j, :, :], k2[:])

                # ---- v + gate*ve (gate = 3*sigmoid, the 3 is folded in ve3).
                # g = x[:, :12] @ Wg^T is tiny (|g| < ~0.5 since Wg ~ 0.02),
                # so sigmoid(g) ~= 0.5 + g*(0.25 - g^2/48), err < 2e-4.
                gg2 = stA.tile([128, 2], f32, name="gg2", tag="gg2")
                nc.scalar.activation(gg2[:], kvg[:, 256:258], Square)
                nc.vector.tensor_scalar(gg2[:], gg2[:], -1.0 / 48.0, 0.25,
                                        mult_op, add_op)
                sg = stA.tile([128, 2], f32, name="sg", tag="sg")
                nc.vector.tensor_mul(sg[:], gg2[:], kvg[:, 256:258])
                nc.vector.tensor_scalar_add(sg[:], sg[:], 0.5)
                gve = stA.tile([128, 2, 64], bf16, name="gve", tag="gve")
                for gg in range(2):
                    nc.gpsimd.tensor_scalar_mul(gve[:, gg, :],
                                                ve_sb[:, j, gg, :],
                                                sg[:, gg:gg + 1])
                nc.vector.tensor_add(
                    v_sb[:, j, :, 0:64],
                    kvg[:, 128:256].rearrange("p (g d) -> p g d", g=2), gve[:])

                # ---- q rms (pre-rope), rstd_q folded with QK_SCALE
                q2 = stA.tile([128, 8, 64], bf16, name="q2", tag="q2")
                nc.scalar.activation(q2[:], q_ps[:], Square)
                msqq = stA.tile([128, 8], f32, name="msqq", tag="msqq")
                nc.vector.tensor_reduce(msqq[:], q2[:], mybir.AxisListType.X,
                                        add_op)
                sqq = stA.tile([128, 8], f32, name="sqq", tag="sqq")
                nc.scalar.activation(sqq[:], msqq[:], Sqrt,
                                     scale=1.0 / (D * QK_SCALE * QK_SCALE),
                                     bias=epsq_sb[:])
                rsq = stA.tile([128, 8], f32, name="rsq", tag="rsq")
                nc.vector.reciprocal(rsq[:], sqq[:])
                # ---- q: PSUM->SBUF bf16 with per-head rstd_q scale
                qs = stA.tile([128, 8, 64], bf16, name="qs", tag="qs")
                for h8 in range(8):
                    nc.vector.tensor_scalar_mul(qs[:, h8, :], q_ps[:, h8, :],
                                                rsq[:, h8:h8 + 1])
                # ---- q rope (muls on Pool, shift-adds on DVE)
                spat_j8 = spat_sb[:, j:j + 1, :].broadcast_to([128, 8, 64])
                cpat_j8 = cpat_sb[:, j:j + 1, :].broadcast_to([128, 8, 64])
                qu = stA.tile([128, 8, 64], bf16, name="qu", tag="qu")
                nc.gpsimd.tensor_mul(qu[:], qs[:], spat_j8)
                qr = stA.tile([128, 8, 64], bf16, name="qr", tag="qr")
                nc.gpsimd.tensor_mul(qr[:], qs[:], cpat_j8)
                nc.vector.tensor_add(qr[:, :, 0:32], qr[:, :, 0:32],
                                     qu[:, :, 32:64])
                nc.vector.tensor_add(qr[:, :, 32:64], qr[:, :, 32:64],
                                     qu[:, :, 0:32])

                pending.append((j, qr, kr))
            emit_transpose(*pending.pop())

        # ================= Stage B + C: attention + out projection ==========
        with tc.tile_pool(name="stB", bufs=2) as stB, \
             tc.tile_pool(name="pB_", bufs=1, space="PSUM") as pB_:
            pkeep = {(r, j): stB.tile([128, 2, 384], bf16, name=f"pk{r}{j}",
                                      tag=f"pk{r}{j}", bufs=1)
                     for r in range(4) for j in (2, 3)}
            ccount = 0
            for h in range(2):
                tsl = slice(512 * h, 512 * h + 512)
                jlist = list(range(0, 4)) if h == 0 else list(range(2, 8))
                for r in range(4):
                    y_ps = pB_.tile([65, 2, 512], f32, name="y_ps", tag="yps",
                                    bufs=2)
                    pvq = []  # PV lags QK/exp/mask by one block

                    def emit_pv(j, p2, first, last):
                        w = min(384, T - 128 * j)
                        a = max(128 * j, 512 * h)
                        b = min(128 * j + w, 512 * h + 512)
                        n0, nn = a - 128 * j, b - a
                        for gg in range(2):
                            nc.tensor.matmul(
                                y_ps[:, gg, a - 512 * h:b - 512 * h],
                                v_sb[:, j, gg, :], p2[:, gg, n0:n0 + nn],
                                start=first, stop=last,
                                skip_group_check=True)

                    for j in jlist:
                        w = min(384, T - 128 * j)
                        if h == 1 and j in (2, 3):
                            p2 = pkeep[(r, j)]  # cached from h == 0
                        else:
                            p2 = (pkeep[(r, j)] if j in (2, 3) else
                                  stB.tile([128, 2, 384], bf16, name="p2",
                                           tag="p2", bufs=3))
                            sc2 = pB_.tile([128, 2, 512], f32, name="sc2",
                                           tag="sc", bufs=2)
                            for gg in range(2):
                                nc.tensor.matmul(
                                    sc2[:, gg, 0:w],
                                    kTf[ts(gg, 64), ts(j, 128)],
                                    qTf[ts(gg, 64), r, 128 * j:128 * j + w],
                                    start=True, stop=True)
                            nc.scalar.activation(p2[:, :, 0:w], sc2[:, :, 0:w],
                                                 Exp)
                            # band edges: causal lower, window upper triangle
                            nc.vector.tensor_mul(
                                p2[:, :, 0:128], p2[:, :, 0:128],
                                mlo_sb[:].unsqueeze(1).broadcast_to([128, 2, 128]))
                            if w > 256:
                                nc.vector.tensor_mul(
                                    p2[:, :, 256:384], p2[:, :, 256:384],
                                    mhi_sb[:].unsqueeze(1).broadcast_to([128, 2, 128]))
                        pvq.append((j, p2))
                        if len(pvq) > 1:
                            jp, pp = pvq.pop(0)
                            emit_pv(jp, pp, jp == jlist[0], False)
                    jp, pp = pvq.pop(0)
                    emit_pv(jp, pp, jp == jlist[0], True)
                    # normalize: 1/sums (row 64) -> rank-1 broadcast -> apply
                    su = stB.tile([1, 2, 512], f32, name="su", tag="su")
                    nc.scalar.activation(su[:], y_ps[64:65, :, :], Copy)
                    rs = stB.tile([1, 2, 512], f32, name="rs", tag="rs")
                    nc.vector.reciprocal_approx_fast(rs[:], su[:])
                    rbs = stB.tile([64, 2, 512], f32, name="rbs", tag="rbs")
                    nc.gpsimd.partition_broadcast(rbs[:], rs[:], channels=64)
                    if debug and h == 0 and r == 0:
                        sums_sb = stB.tile([1, 2, 512], f32, name="sums_sb",
                                           tag="sums_sb")
                        nc.vector.tensor_copy(sums_sb[:], y_ps[64:65, :, :])
                        nc.sync.dma_start(dbg["d_sums"][:], sums_sb[:])
                        nc.sync.dma_start(dbg["d_rs"][:], rs[:])
                        nc.sync.dma_start(dbg["d_rbs"][:], rbs[:])
                    nc.vector.tensor_mul(yTf[0:64, r, tsl], y_ps[0:64, 0, :],
                                         rbs[:, 0, :])
                    nc.vector.tensor_mul(yTf[64:128, r, tsl], y_ps[0:64, 1, :],
                                         rbs[:, 1, :])

                # ---- stage C for this half (PE overlaps next half's B).
                # o_ps shares the "sc" buffer rotation to stay within 8 banks.
                for ct in range(8):
                    o_full = pB_.tile([128, 2, 512], f32, name="sc2", tag="sc",
                                      bufs=2)
                    o_ps = o_full[:, 0, :]
                    for kr in range(4):
                        nc.tensor.matmul(o_ps, wo_sb[:, kr, ts(ct, 128)],
                                         yTf[:, kr, tsl], start=(kr == 0),
                                         stop=(kr == 3))
                    o_sb = stB.tile([128, 512], bf16, name="o_sb", tag="osb",
                                    bufs=3)
                    if ccount % 2 == 0:
                        nc.scalar.activation(o_sb[:], o_ps, Copy)
                    else:
                        nc.vector.tensor_copy(o_sb[:], o_ps)
                    qdma = nc.sync if ccount % 2 == 0 else nc.scalar
                    qdma.dma_start(outT[ts(ct, 128), tsl], o_sb[:])
                    ccount += 1

        if debug:
            nc.sync.dma_start(dbg["d_qTf"][:], qTf[:])
            nc.sync.dma_start(dbg["d_kTf"][:], kTf[:])
            nc.sync.dma_start(dbg["d_v"][:], v_sb[:])
            nc.sync.dma_start(dbg["d_yTf"][:], yTf[:])

    nc.compile()
    return nc


def _prep_core_inputs(x, ve3, cosT, sinT, Wq, Wk, Wv, Wo, Wg, consts, b, s):
    """Host-side arrangement of one core's DRAM inputs (bf16)."""
    import ml_dtypes
    bf = ml_dtypes.bfloat16
    g0, g1 = 2 * s, 2 * s + 1

    # xtn[p, kc, j, tt] = x[b, 128j+tt, 128kc+p]
    xtn = np.ascontiguousarray(
        x[b].reshape(8, 128, 8, 128).transpose(3, 2, 0, 1)).astype(bf)

    Wq4 = Wq.reshape(HKV, REP, D, C)
    # wq_cols[c, r*128+gg*64+d] = Wq4[2s+gg, r, d, c]
    wq_cols = np.concatenate(
        [Wq4[g, r].T for r in range(REP) for g in (g0, g1)],
        axis=1)                                            # (C, 512)
    wqn = np.ascontiguousarray(
        wq_cols.reshape(8, 128, 512).transpose(1, 0, 2)).astype(bf)

    Wk3 = Wk.reshape(HKV, D, C)
    wk_cols = np.concatenate([Wk3[g0].T, Wk3[g1].T], axis=1)  # (C, 128)
    wkn = np.ascontiguousarray(
        wk_cols.reshape(8, 128, 128).transpose(1, 0, 2)).astype(bf)
    Wv3 = Wv.reshape(HKV, D, C)
    wv_cols = np.concatenate([Wv3[g0].T, Wv3[g1].T], axis=1)
    wvn = np.ascontiguousarray(
        wv_cols.reshape(8, 128, 128).transpose(1, 0, 2)).astype(bf)

    Wo4 = Wo.reshape(C, HKV, REP, D)
    # won[gg*64+d, r, c] = Wo4[c, 2s+gg, r, d]
    won = np.ascontiguousarray(
        np.stack([np.concatenate([Wo4[:, g0, r, :].T, Wo4[:, g1, r, :].T],
                                 axis=0) for r in range(REP)],
                 axis=1)).astype(bf)                       # (128, 4, C)

    wgn = np.zeros((16, 2), dtype=np.float32)
    wgn[0:GATE_CH, 0] = Wg[g0]
    wgn[0:GATE_CH, 1] = Wg[g1]
    wgn = wgn.astype(bf)

    ve4 = ve3[b].reshape(T, HKV, D)
    ve3n = np.ascontiguousarray(
        np.stack([ve4[:, g0, :], ve4[:, g1, :]],
                 axis=1).reshape(8, 128, 2, 64).transpose(1, 0, 2, 3)).astype(bf)

    d = dict(xtn=xtn, wqn=wqn, wkn=wkn, wvn=wvn, won=won, wgn=wgn, ve3n=ve3n)
    d.update(consts)
    return d


def _const_inputs(cosT, sinT):
    import ml_dtypes
    bf = ml_dtypes.bfloat16
    # cpat[t, d] = cos[t, d % 32]; spat[t, d] = -sin[t,d] (d<32) else sin[t,d-32]
    cfull = np.concatenate([cosT, cosT], axis=1)           # (T, 64)
    sfull = np.concatenate([-sinT, sinT], axis=1)          # (T, 64)
    cpat = np.ascontiguousarray(
        cfull.reshape(8, 128, 64).transpose(1, 0, 2)).astype(bf)
    spat = np.ascontiguousarray(
        sfull.reshape(8, 128, 64).transpose(1, 0, 2)).astype(bf)
    idx = np.arange(128)
    mlo = (idx[None, :] >= idx[:, None]).astype(bf)        # keep col >= row
    mhi = (idx[None, :] <= idx[:, None]).astype(bf)        # keep col <= row
    return dict(cpat=cpat, spat=spat, mlo=mlo, mhi=mhi)


def kernel(x, ve, cos, sin, Wq, Wk, Wv, Wo, Wg, window_size):
    from concourse.bass_utils import run_bass_kernel_spmd

    assert int(window_size) == WINDOW
    x = np.asarray(x, dtype=np.float32)
    ve3 = 3.0 * np.asarray(ve, dtype=np.float32)
    Wq = np.asarray(Wq, dtype=np.float32)
    Wk = np.asarray(Wk, dtype=np.float32)
    Wv = np.asarray(Wv, dtype=np.float32)
    Wo = np.asarray(Wo, dtype=np.float32)
    Wg = np.asarray(Wg, dtype=np.float32)
    cosT = np.asarray(cos, dtype=np.float32).reshape(T, D // 2)
    sinT = np.asarray(sin, dtype=np.float32).reshape(T, D // 2)
    consts = _const_inputs(cosT, sinT)

    if "nc" not in _CACHE:
        _CACHE["nc"] = _build_program()
    nc = _CACHE["nc"]

    in_maps = []
    for core in range(NCORES):
        b, s = core // 2, core % 2
        in_maps.append(_prep_core_inputs(x, ve3, cosT, sinT,
                                         Wq, Wk, Wv, Wo, Wg, consts, b, s))

    res = run_bass_kernel_spmd(nc, in_maps, core_ids=list(range(NCORES)))
    out = np.empty((B, T, C), dtype=np.float32)
    for b in range(B):
        acc = (res.results[2 * b]["out_t"].astype(np.float32) +
               res.results[2 * b + 1]["out_t"].astype(np.float32))
        out[b] = acc.T
    return out


# revision 53
# speedup vs baseline: 1.1928x; 1.0780x over previous
"""Sliding-window GQA causal self-attention for Trainium2, 8 NeuronCores.

Sharding: 8 cores = 4 batches x 2 head-shards. Each core handles one batch
and 2 of the 4 KV groups (8 of 16 Q heads). Core computes a full [C, T]
partial of the output projection; host sums the two shards per batch.

Design (v2, natural-layout stage A, bf16 operands):
  Projections run "natural" (tokens on partitions): per 128-token block j,
  q_ps[t, 512], k/v/gate packed in one PSUM tile. RoPE becomes free-dim
  32-shifts (no partition swaps); rms uses rotation invariance (computed
  from pre-rope PSUM via square+reduce); rstd_k folds into the PSUM->SBUF
  copy (Act copy with per-partition scale), rstd_q (with the 1.2*1.2/8
  score scale) folds into the per-head Act copies. q/k transpose via PE
  into bf16 PSUM, Pool copies them out.

  Attention: per (h-half, r): scores^T via bf16 QK matmuls, Exp on Act,
  band-edge masks via DVE multiplies with precomputed triangle masks.
  V carries a ones-column so softmax denominators fall out of the PV
  matmul (row 64 of y_ps). Normalize: DVE reciprocal -> rank-1 broadcast
  matmul -> DVE/Pool multiplies into yTf (bf16). Output projection per
  h-half interleaves with the other half's attention on the PE.
"""
import numpy as np

B, T, C = 4, 1024, 1024
H, HKV, D = 16, 4, 64
REP = H // HKV
WINDOW = 256
GATE_CH = 12
NCORES = 8
EPS = float(np.finfo(np.float32).eps)
QK_SCALE = 1.2 * 1.2 / 8.0  # two rms gains (1.2 each) * 1/sqrt(D)

_CACHE = {}


def _build_program(debug=False, reps=1):
    from contextlib import ExitStack
    import concourse.bass as bass
    import concourse.tile as tile
    from concourse import bacc, mybir
    from concourse.masks import make_identity

    f32 = mybir.dt.float32
    f32r = mybir.dt.float32r
    bf16 = mybir.dt.bfloat16
    ts = bass.ts

    nc = bacc.Bacc("TRN2", target_bir_lowering=False, debug=False,
                   enable_asserts=True, num_devices=NCORES)

    def din(name, shape, dt=bf16):
        return nc.dram_tensor(name, shape, dt, kind="ExternalInput").ap()

    # host-prearranged layouts (see _prep_core_inputs)
    xtn = din("xtn", [128, 8, 8, 128])    # [c%128, c//128, j, t%128] = x^T
    wqn = din("wqn", [128, 8, 512])       # [c%128, c//128, r*128+gg*64+d]
    wkn = din("wkn", [128, 8, 128])       # [c%128, c//128, gg*64+d]
    wvn = din("wvn", [128, 8, 128])
    won = din("won", [128, 4, 1024])      # [gg*64+d, r, c]
    wgn = din("wgn", [16, 2])             # zero-padded 12->16 gate rows
    ve3n = din("ve3n", [128, 8, 2, 64])   # [t%128, j, gg, d] = 3*ve
    cpat = din("cpat", [128, 8, 64])      # cos[t, d%32]
    spat = din("spat", [128, 8, 64])      # -sin[t,d] / +sin[t,d-32] halves
    mlo = din("mlo", [128, 128])          # causal edge: 1 if col >= row
    mhi = din("mhi", [128, 128])          # window edge: 1 if col <= row
    outT = nc.dram_tensor("out_t", [C, T], bf16, kind="ExternalOutput").ap()
    dbg = {}
    if debug:
        for nm, shp in [("d_qTf", [128, 4, T]), ("d_kTf", [128, T]),
                        ("d_v", [128, 8, 2, 65]), ("d_yTf", [128, 4, T]),
                        ("d_ks", [128, 8, 2, 64]), ("d_kr", [128, 8, 2, 64]),
                        ("d_rsk", [128, 8, 2]), ("d_msqk", [128, 8, 2]),
                        ("d_kraw", [128, 8, 2, 64]), ("d_k2", [128, 8, 2, 64]),
                        ("d_rs", [1, 2, 512]), ("d_rbs", [64, 2, 512]),
                        ("d_sums", [1, 2, 512])]:
            dt = f32 if nm in ("d_rsk", "d_msqk", "d_rs", "d_rbs",
                               "d_sums") else bf16
            dbg[nm] = nc.dram_tensor(nm, shp, dt, kind="ExternalOutput").ap()

    Exp = mybir.ActivationFunctionType.Exp
    Square = mybir.ActivationFunctionType.Square
    Sqrt = mybir.ActivationFunctionType.Sqrt
    Copy = mybir.ActivationFunctionType.Copy
    add_op = mybir.AluOpType.add
    mult_op = mybir.AluOpType.mult

    with tile.TileContext(nc) as tc:
     for _rep in range(reps):
      with ExitStack() as ctx:
        sing = ctx.enter_context(tc.tile_pool(name="sing", bufs=1))

        # ---------- persistent tiles + loads ----------
        # SP queue: xt0/wk/wv first so tblock-0 k/v matmuls start ASAP;
        # Act queue takes the rope/gate tables + wo (needed later).
        xt_sb = [sing.tile([128, 8, 128], bf16, name=f"xt{j}") for j in range(8)]
        nc.sync.dma_start(xt_sb[0][:], xtn[:, :, 0, :])
        wk_sb = sing.tile([128, 8, 128], bf16, name="wk_sb")
        nc.sync.dma_start(wk_sb[:], wkn[:])
        wv_sb = sing.tile([128, 8, 128], bf16, name="wv_sb")
        nc.sync.dma_start(wv_sb[:], wvn[:])
        wg_sb = sing.tile([16, 2], bf16, name="wg_sb")
        nc.sync.dma_start(wg_sb[:], wgn[:])
        nc.sync.dma_start(xt_sb[1][:], xtn[:, :, 1, :])
        wq_sb = sing.tile([128, 8, 512], bf16, name="wq_sb")
        nc.sync.dma_start(wq_sb[:, 0:4, :], wqn[:, 0:4, :])
        nc.sync.dma_start(wq_sb[:, 4:8, :], wqn[:, 4:8, :])
        for j in range(2, 8):
            nc.sync.dma_start(xt_sb[j][:], xtn[:, :, j, :])
        cpat_sb = sing.tile([128, 8, 64], bf16, name="cpat_sb")
        nc.scalar.dma_start(cpat_sb[:], cpat[:])
        spat_sb = sing.tile([128, 8, 64], bf16, name="spat_sb")
        nc.scalar.dma_start(spat_sb[:], spat[:])
        ve_sb = sing.tile([128, 8, 2, 64], bf16, name="ve_sb")
        nc.scalar.dma_start(ve_sb[:], ve3n[:])
        mlo_sb = sing.tile([128, 128], bf16, name="mlo_sb")
        nc.scalar.dma_start(mlo_sb[:], mlo[:])
        mhi_sb = sing.tile([128, 128], bf16, name="mhi_sb")
        nc.scalar.dma_start(mhi_sb[:], mhi[:])
        # wo only needed by stage C
        wo_sb = sing.tile([128, 4, 1024], bf16, name="wo_sb")
        nc.scalar.dma_start(wo_sb[:], won[:])

        ident = sing.tile([128, 128], bf16, name="ident")
        make_identity(nc, ident[:])

        epsk_sb = sing.tile([128, 1], f32, name="epsk_sb")
        nc.vector.memset(epsk_sb[:], EPS)
        epsq_sb = sing.tile([128, 1], f32, name="epsq_sb")
        nc.vector.memset(epsq_sb[:], EPS / (QK_SCALE * QK_SCALE))
        # persistent tensors split by T-halves so stage B can start on the
        # first half while stage A still works on the second
        qTf_h = [sing.tile([128, 4, 512], bf16, name=f"qTf{i}") for i in range(2)]
        kTf_h = [sing.tile([128, 512], bf16, name=f"kTf{i}") for i in range(2)]
        v_h = [sing.tile([128, 4, 2, 65], bf16, name=f"v{i}") for i in range(2)]
        yTf_h = [sing.tile([128, 4, 512], bf16, name=f"yTf{i}") for i in range(2)]
        nc.vector.memset(v_h[0][:, :, :, 64:65], 1.0)
        nc.vector.memset(v_h[1][:, :, :, 64:65], 1.0)

        # ================= Stage A: projections / rope / rms / gate =========
        # 3-stage software pipeline: kvg matmuls for block j ride ahead of
        # q matmuls for j-1 and transposes for j-2 so the PE never stalls
        # on the vector chains or the wq DMA.
        with tc.tile_pool(name="stA", bufs=2) as stA, \
             tc.tile_pool(name="pA_", bufs=2, space="PSUM") as pA_:

            def emit_kvg(j):
                xt = xt_sb[j]
                # k, v, gate groups must be sequential: interleaving two
                # accumulation groups in one PSUM bank loses contributions.
                kvg = pA_.tile([128, 258], f32, name="kvg", tag="kvg")
                for kc in range(8):
                    nc.tensor.matmul(kvg[:, 0:128], xt[:, kc, :],
                                     wk_sb[:, kc, :], start=(kc == 0),
                                     stop=(kc == 7), skip_group_check=True)
                for kc in range(8):
                    nc.tensor.matmul(kvg[:, 128:256], xt[:, kc, :],
                                     wv_sb[:, kc, :], start=(kc == 0),
                                     stop=(kc == 7), skip_group_check=True)
                nc.tensor.matmul(kvg[:, 256:258], xt[0:16, 0, :], wg_sb[:],
                                 start=True, stop=True, skip_group_check=True)
                return kvg

            def emit_qproj(j):
                xt = xt_sb[j]
                q_ps = pA_.tile([128, 8, 64], f32, name="q_ps", tag="qps")
                for kc in range(8):
                    nc.tensor.matmul(q_ps[:].rearrange("p h d -> p (h d)"),
                                     xt[:, kc, :], wq_sb[:, kc, :],
                                     start=(kc == 0), stop=(kc == 7))
                return q_ps

            def emit_chain(j, kvg, q_ps):
                kk = kvg[:, 0:128].rearrange("p (g d) -> p g d", g=2)
                # ---- k rms (pre-rope; rotation preserves norms).
                k2 = stA.tile([128, 2, 64], bf16, name="k2", tag="k2")
                nc.scalar.activation(k2[:], kk, Square)
                msqk = stA.tile([128, 2], f32, name="msqk", tag="msqk")
                nc.vector.tensor_reduce(msqk[:], k2[:], mybir.AxisListType.X,
                                        add_op)
                sqk = stA.tile([128, 2], f32, name="sqk", tag="sqk")
                nc.scalar.activation(sqk[:], msqk[:], Sqrt, scale=1.0 / D,
                                     bias=epsk_sb[:])
                rsk = stA.tile([128, 2], f32, name="rsk", tag="rsk")
                nc.vector.reciprocal(rsk[:], sqk[:])
                # ---- k: PSUM->SBUF with rstd_k folded into the copy
                ks = stA.tile([128, 2, 64], bf16, name="ks", tag="ks")
                for gg in range(2):
                    nc.scalar.activation(ks[:, gg, :], kk[:, gg, :], Copy,
                                         scale=rsk[:, gg:gg + 1])
                # ---- k rope: kr = ks*cpat + shift32(ks*spat)
                spat_j2 = spat_sb[:, j:j + 1, :].broadcast_to([128, 2, 64])
                cpat_j2 = cpat_sb[:, j:j + 1, :].broadcast_to([128, 2, 64])
                ku = stA.tile([128, 2, 64], bf16, name="ku", tag="ku")
                nc.gpsimd.tensor_mul(ku[:], ks[:], spat_j2)
                kr = stA.tile([128, 2, 64], bf16, name="kr", tag="kr")
                nc.gpsimd.tensor_mul(kr[:], ks[:], cpat_j2)
                nc.vector.tensor_add(kr[:, :, 0:32], kr[:, :, 0:32],
                                     ku[:, :, 32:64])
                nc.vector.tensor_add(kr[:, :, 32:64], kr[:, :, 32:64],
                                     ku[:, :, 0:32])

                # ---- v + gate*ve (gate = 3*sigmoid, 3 folded in ve3).
                # g = x[:, :12] @ Wg^T is tiny (|g| < ~0.5 since Wg ~ 0.02):
                # sigmoid(g) ~= 0.5 + g*(0.25 - g^2/48), err < 2e-4.
                gg2 = stA.tile([128, 2], f32, name="gg2", tag="gg2")
                nc.scalar.activation(gg2[:], kvg[:, 256:258], Square)
                nc.vector.tensor_scalar(gg2[:], gg2[:], -1.0 / 48.0, 0.25,
                                        mult_op, add_op)
                sg = stA.tile([128, 2], f32, name="sg", tag="sg")
                nc.vector.tensor_mul(sg[:], gg2[:], kvg[:, 256:258])
                nc.vector.tensor_scalar_add(sg[:], sg[:], 0.5)
                gve = stA.tile([128, 2, 64], bf16, name="gve", tag="gve")
                for gg in range(2):
                    nc.gpsimd.tensor_scalar_mul(gve[:, gg, :],
                                                ve_sb[:, j, gg, :],
                                                sg[:, gg:gg + 1])
                nc.vector.tensor_add(
                    v_h[j // 4][:, j % 4, :, 0:64],
                    kvg[:, 128:256].rearrange("p (g d) -> p g d", g=2), gve[:])

                # ---- q rms (pre-rope), rstd_q folded with QK_SCALE
                q2 = stA.tile([128, 8, 64], bf16, name="q2", tag="q2")
                nc.scalar.activation(q2[:], q_ps[:], Square)
                msqq = stA.tile([128, 8], f32, name="msqq", tag="msqq")
                nc.vector.tensor_reduce(msqq[:], q2[:], mybir.AxisListType.X,
                                        add_op)
                sqq = stA.tile([128, 8], f32, name="sqq", tag="sqq")
                nc.scalar.activation(sqq[:], msqq[:], Sqrt,
                                     scale=1.0 / (D * QK_SCALE * QK_SCALE),
                                     bias=epsq_sb[:])
                rsq = stA.tile([128, 8], f32, name="rsq", tag="rsq")
                nc.vector.reciprocal(rsq[:], sqq[:])
                # ---- q: PSUM->SBUF bf16 with per-head rstd_q scale
                qs = stA.tile([128, 8, 64], bf16, name="qs", tag="qs")
                for h8 in range(8):
                    nc.vector.tensor_scalar_mul(qs[:, h8, :], q_ps[:, h8, :],
                                                rsq[:, h8:h8 + 1])
                # ---- q rope (muls on Pool, shift-adds on DVE)
                spat_j8 = spat_sb[:, j:j + 1, :].broadcast_to([128, 8, 64])
                cpat_j8 = cpat_sb[:, j:j + 1, :].broadcast_to([128, 8, 64])
                qu = stA.tile([128, 8, 64], bf16, name="qu", tag="qu")
                nc.gpsimd.tensor_mul(qu[:], qs[:], spat_j8)
                qr = stA.tile([128, 8, 64], bf16, name="qr", tag="qr")
                nc.gpsimd.tensor_mul(qr[:], qs[:], cpat_j8)
                nc.vector.tensor_add(qr[:, :, 0:32], qr[:, :, 0:32],
                                     qu[:, :, 32:64])
                nc.vector.tensor_add(qr[:, :, 32:64], qr[:, :, 32:64],
                                     qu[:, :, 0:32])
                if debug:
                    nc.sync.dma_start(dbg["d_ks"][:, j, :, :], ks[:])
                    nc.sync.dma_start(dbg["d_kr"][:, j, :, :], kr[:])
                    nc.sync.dma_start(dbg["d_rsk"][:, j, :], rsk[:])
                return qr, kr

            def emit_transpose(j, qr, kr):
                tp = pA_.tile([128, 640], bf16, name="tp", tag="tp")
                for r in range(4):
                    nc.tensor.transpose(
                        tp[:, ts(r, 128)],
                        qr[:, 2 * r:2 * r + 2, :].rearrange("p g d -> p (g d)"),
                        ident[:])
                nc.tensor.transpose(
                    tp[:, 512:640], kr[:].rearrange("p g d -> p (g d)"),
                    ident[:])
                nc.scalar.activation(
                    qTf_h[j // 4][:, :, ts(j % 4, 128)],
                    tp[:, 0:512].rearrange("p (r t) -> p r t", r=4), Copy)
                nc.vector.tensor_copy(kTf_h[j // 4][:, ts(j % 4, 128)],
                                      tp[:, 512:640])

            kvg_q, chain_q = [], []
            for j in range(10):
                if j < 8:
                    kvg_q.append((j, emit_kvg(j)))
                if kvg_q and j >= 1:
                    jj, kvg = kvg_q.pop(0)
                    q_ps = emit_qproj(jj)
                    chain_q.append((jj, emit_chain(jj, kvg, q_ps)))
                if chain_q and j >= 2:
                    jj, (qr, kr) = chain_q.pop(0)
                    emit_transpose(jj, qr, kr)

        # ================= Stage B + C: attention + out projection ==========
        with tc.tile_pool(name="stB", bufs=2) as stB, \
             tc.tile_pool(name="pB_", bufs=1, space="PSUM") as pB_:
            pkeep = {(r, j): stB.tile([128, 2, 384], bf16, name=f"pk{r}{j}",
                                      tag=f"pk{r}{j}", bufs=1)
                     for r in range(4) for j in (2, 3)}
            ccount = 0

            def emit_cstage(h, cts):
                nonlocal ccount
                tsl = slice(512 * h, 512 * h + 512)
                for ct in cts:
                    # o_ps shares the "sc" rotation to stay within 8 banks
                    o_full = pB_.tile([128, 2, 512], f32, name="sc2", tag="sc",
                                      bufs=2)
                    o_ps = o_full[:, 0, :]
                    for kr in range(4):
                        nc.tensor.matmul(o_ps, wo_sb[:, kr, ts(ct, 128)],
                                         yTf_h[h][:, kr, :], start=(kr == 0),
                                         stop=(kr == 3))
                    o_sb = stB.tile([128, 512], bf16, name="o_sb", tag="osb",
                                    bufs=3)
                    if ccount % 2 == 0:
                        nc.scalar.activation(o_sb[:], o_ps, Copy)
                    else:
                        nc.vector.tensor_copy(o_sb[:], o_ps)
                    qdma = nc.sync if ccount % 2 == 0 else nc.scalar
                    qdma.dma_start(outT[ts(ct, 128), tsl], o_sb[:])
                    ccount += 1

            for h in range(2):
                jlist = list(range(0, 4)) if h == 0 else list(range(2, 8))
                for r in range(4):
                    y_ps = pB_.tile([65, 2, 512], f32, name="y_ps", tag="yps",
                                    bufs=2)
                    pvq = []  # PV lags QK/exp/mask by two blocks

                    def emit_pv(j, p2, first, last):
                        w = min(384, T - 128 * j)
                        a = max(128 * j, 512 * h)
                        b = min(128 * j + w, 512 * h + 512)
                        n0, nn = a - 128 * j, b - a
                        for gg in range(2):
                            nc.tensor.matmul(
                                y_ps[:, gg, a - 512 * h:b - 512 * h],
                                v_h[j // 4][:, j % 4, gg, :],
                                p2[:, gg, n0:n0 + nn],
                                start=first, stop=last,
                                skip_group_check=True)

                    for j in jlist:
                        w = min(384, T - 128 * j)
                        if h == 1 and j in (2, 3):
                            p2 = pkeep[(r, j)]  # cached from h == 0
                        else:
                            p2 = (pkeep[(r, j)] if j in (2, 3) else
                                  stB.tile([128, 2, 384], bf16, name="p2",
                                           tag="p2", bufs=3))
                            sc2 = pB_.tile([128, 2, 512], f32, name="sc2",
                                           tag="sc", bufs=2)
                            for gg in range(2):
                                # QK; q columns may straddle the T-half split
                                c0 = 128 * j
                                while c0 < 128 * j + w:
                                    c1 = min(128 * j + w,
                                             (c0 // 512 + 1) * 512)
                                    nc.tensor.matmul(
                                        sc2[:, gg, c0 - 128 * j:c1 - 128 * j],
                                        kTf_h[j // 4][ts(gg, 64),
                                                      ts(j % 4, 128)],
                                        qTf_h[c0 // 512][ts(gg, 64), r,
                                                         c0 % 512:
                                                         c0 % 512 + c1 - c0],
                                        start=True, stop=True,
                                        skip_group_check=True)
                                    c0 = c1
                            nc.scalar.activation(p2[:, :, 0:w], sc2[:, :, 0:w],
                                                 Exp)
                            # band edges: causal lower tri on Pool,
                            # window upper tri on DVE
                            nc.gpsimd.affine_select(
                                p2[:, :, 0:128], p2[:, :, 0:128],
                                compare_op=mybir.AluOpType.is_ge, fill=0.0,
                                base=0, pattern=[[0, 2], [1, 128]],
                                channel_multiplier=-1)
                            if w > 256:
                                nc.vector.tensor_mul(
                                    p2[:, :, 256:384], p2[:, :, 256:384],
                                    mhi_sb[:].unsqueeze(1).broadcast_to(
                                        [128, 2, 128]))
                        pvq.append((j, p2))
                        if len(pvq) > 2:
                            jp, pp = pvq.pop(0)
                            emit_pv(jp, pp, jp == jlist[0], False)
                    while pvq:
                        jp, pp = pvq.pop(0)
                        emit_pv(jp, pp, jp == jlist[0], len(pvq) == 0)
                    # normalize: 1/sums (row 64) -> broadcast -> apply
                    su = stB.tile([1, 2, 512], f32, name="su", tag="su")
                    nc.scalar.activation(su[:], y_ps[64:65, :, :], Copy)
                    rs = stB.tile([1, 2, 512], f32, name="rs", tag="rs")
                    nc.vector.reciprocal_approx_fast(rs[:], su[:])
                    rbs = stB.tile([64, 2, 512], f32, name="rbs", tag="rbs")
                    nc.gpsimd.partition_broadcast(rbs[:], rs[:], channels=64)
                    nc.vector.tensor_mul(yTf_h[h][0:64, r, :],
                                         y_ps[0:64, 0, :], rbs[:, 0, :])
                    nc.vector.tensor_mul(yTf_h[h][64:128, r, :],
                                         y_ps[0:64, 1, :], rbs[:, 1, :])
                    # interleave the previous half's output projection
                    if h == 1:
                        emit_cstage(0, range(2 * r, 2 * r + 2))
            emit_cstage(1, range(8))

        if debug:
            nc.sync.dma_start(dbg["d_qTf"][:], qTf[:])
            nc.sync.dma_start(dbg["d_kTf"][:], kTf[:])
            nc.sync.dma_start(dbg["d_v"][:], v_sb[:])
            nc.sync.dma_start(dbg["d_yTf"][:], yTf[:])

    nc.compile()
    return nc


def _prep_core_inputs(x, ve3, cosT, sinT, Wq, Wk, Wv, Wo, Wg, consts, b, s):
    """Host-side arrangement of one core's DRAM inputs (bf16)."""
    import ml_dtypes
    bf = ml_dtypes.bfloat16
    g0, g1 = 2 * s, 2 * s + 1

    # xtn[p, kc, j, tt] = x[b, 128j+tt, 128kc+p]
    xtn = np.ascontiguousarray(
        x[b].reshape(8, 128, 8, 128).transpose(3, 2, 0, 1)).astype(bf)

    Wq4 = Wq.reshape(HKV, REP, D, C)
    # wq_cols[c, r*128+gg*64+d] = Wq4[2s+gg, r, d, c]
    wq_cols = np.concatenate(
        [Wq4[g, r].T for r in range(REP) for g in (g0, g1)],
        axis=1)                                            # (C, 512)
    wqn = np.ascontiguousarray(
        wq_cols.reshape(8, 128, 512).transpose(1, 0, 2)).astype(bf)

    Wk3 = Wk.reshape(HKV, D, C)
    wk_cols = np.concatenate([Wk3[g0].T, Wk3[g1].T], axis=1)  # (C, 128)
    wkn = np.ascontiguousarray(
        wk_cols.reshape(8, 128, 128).transpose(1, 0, 2)).astype(bf)
    Wv3 = Wv.reshape(HKV, D, C)
    wv_cols = np.concatenate([Wv3[g0].T, Wv3[g1].T], axis=1)
    wvn = np.ascontiguousarray(
        wv_cols.reshape(8, 128, 128).transpose(1, 0, 2)).astype(bf)

    Wo4 = Wo.reshape(C, HKV, REP, D)
    # won[gg*64+d, r, c] = Wo4[c, 2s+gg, r, d]
    won = np.ascontiguousarray(
        np.stack([np.concatenate([Wo4[:, g0, r, :].T, Wo4[:, g1, r, :].T],
                                 axis=0) for r in range(REP)],
                 axis=1)).astype(bf)                       # (128, 4, C)

    wgn = np.zeros((16, 2), dtype=np.float32)
    wgn[0:GATE_CH, 0] = Wg[g0]
    wgn[0:GATE_CH, 1] = Wg[g1]
    wgn = wgn.astype(bf)

    ve4 = ve3[b].reshape(T, HKV, D)
    ve3n = np.ascontiguousarray(
        np.stack([ve4[:, g0, :], ve4[:, g1, :]],
                 axis=1).reshape(8, 128, 2, 64).transpose(1, 0, 2, 3)).astype(bf)

    d = dict(xtn=xtn, wqn=wqn, wkn=wkn, wvn=wvn, won=won, wgn=wgn, ve3n=ve3n)
    d.update(consts)
    return d


def _const_inputs(cosT, sinT):
    import ml_dtypes
    bf = ml_dtypes.bfloat16
    # cpat[t, d] = cos[t, d % 32]; spat[t, d] = -sin[t,d] (d<32) else sin[t,d-32]
    cfull = np.concatenate([cosT, cosT], axis=1)           # (T, 64)
    sfull = np.concatenate([-sinT, sinT], axis=1)          # (T, 64)
    cpat = np.ascontiguousarray(
        cfull.reshape(8, 128, 64).transpose(1, 0, 2)).astype(bf)
    spat = np.ascontiguousarray(
        sfull.reshape(8, 128, 64).transpose(1, 0, 2)).astype(bf)
    idx = np.arange(128)
    mlo = (idx[None, :] >= idx[:, None]).astype(bf)        # keep col >= row
    mhi = (idx[None, :] <= idx[:, None]).astype(bf)        # keep col <= row
    return dict(cpat=cpat, spat=spat, mlo=mlo, mhi=mhi)


def kernel(x, ve, cos, sin, Wq, Wk, Wv, Wo, Wg, window_size):
    from concourse.bass_utils import run_bass_kernel_spmd

    assert int(window_size) == WINDOW
    x = np.asarray(x, dtype=np.float32)
    ve3 = 3.0 * np.asarray(ve, dtype=np.float32)
    Wq = np.asarray(Wq, dtype=np.float32)
    Wk = np.asarray(Wk, dtype=np.float32)
    Wv = np.asarray(Wv, dtype=np.float32)
    Wo = np.asarray(Wo, dtype=np.float32)
    Wg = np.asarray(Wg, dtype=np.float32)
    cosT = np.asarray(cos, dtype=np.float32).reshape(T, D // 2)
    sinT = np.asarray(sin, dtype=np.float32).reshape(T, D // 2)
    consts = _const_inputs(cosT, sinT)

    if "nc" not in _CACHE:
        _CACHE["nc"] = _build_program()
    nc = _CACHE["nc"]

    in_maps = []
    for core in range(NCORES):
        b, s = core // 2, core % 2
        in_maps.append(_prep_core_inputs(x, ve3, cosT, sinT,
                                         Wq, Wk, Wv, Wo, Wg, consts, b, s))

    res = run_bass_kernel_spmd(nc, in_maps, core_ids=list(range(NCORES)))
    out = np.empty((B, T, C), dtype=np.float32)
    for b in range(B):
        acc = (res.results[2 * b]["out_t"].astype(np.float32) +
               res.results[2 * b + 1]["out_t"].astype(np.float32))
        out[b] = acc.T
    return out
